# revision 5
# baseline (speedup 1.0000x reference)
"""GCN block (GraphConv + LayerNorm + ReLU + skip projection) on 8 Trainium2 cores.

Strategy (dst-node sharding, per spec sharding_hint):
- 100000 dst nodes -> 784 tiles of 128 dsts (padded to 100352); tiles snake-dealt
  to 8 cores by edge count so every core runs an identical (SPMD) program.
- Edges routed to the core owning their dst tile. Per (tile, src-bank) edge lists
  are padded to multiples of 128; the per-slot/bank edge-tile counts are made
  uniform across cores (max), so one NEFF serves all cores.
- Aggregation agg^T = H^T S via TensorE: H = gathered fp16 src feature rows
  (dma_gather, int16 indices => features split into 4 banks of 25088 rows);
  S[e, d] = norm_src[src_e]*norm_dst[dst_e] * (slot_e == d) built on DVE with one
  fused tensor_scalar(is_equal, mult) against an iota tile.
- gcn = agg @ W + b via fp16 matmul (b folded in with a k=1 ones-row matmul);
  LayerNorm via bn_stats/bn_aggr; skip = features @ skip_W + skip_b in fp32
  (features^T pre-transposed on host); relu + add; one DMA out per 8-slot group.
"""

import sys

sys.path.insert(0, "/opt/trn_rl_repo")

import numpy as np

import concourse.bass as bass  # noqa: F401
import concourse.tile as tile
from concourse import bacc, mybir

# ---------------- problem constants (hardcoded per spec) ----------------
N = 100000
F = 128
HID = 256
NC = 8
TD = 128  # dsts per tile
EPS = 1e-5
NTILES = 784  # ceil(100000/128)=782, padded to a multiple of NC
NP = NTILES * TD  # 100352 padded node space
NB = 4  # src banks (dma_gather idxs are int16)
BS = NP // NB  # 25088 rows per bank
SLOTS = NTILES // NC  # 98 per core
G = 8  # slots per gather group
NGROUPS = (SLOTS + G - 1) // G  # 13
GCH = 1024  # max idxs per dma_gather instruction (Q7 scratch limit)

f16 = mybir.dt.float16
f32 = mybir.dt.float32
i16 = mybir.dt.int16


# ---------------- host-side graph preprocessing ----------------

def _plan(src, dst, opt_iters=600_000):
    """Compute the SPMD-uniform structure: tile->core deal, per (slot, bank)
    edge-tile counts T[s][b], and the flat (group, bank, slot) segment layout.

    Tiles are grouped into slots of NC so that the per-slot/bank max (which all
    cores pad to) is small: snake-deal by total count, then local-search swaps
    minimizing sum_s,b max_c ceil(cnt/128). Deterministic (fixed iteration
    count) so repeated runs produce identical programs and hit the NEFF cache."""
    tile_id = dst // TD
    bank = src // BS

    cnt = np.zeros((NTILES, NB), dtype=np.int64)
    np.add.at(cnt, (tile_id, bank), 1)
    tot = cnt.sum(1)

    # snake-deal tiles (desc by edge count) to slot groups
    order = np.argsort(-tot, kind="stable")
    arr = np.empty((SLOTS, NC), dtype=np.int64)
    for i, t in enumerate(order):
        r, j = divmod(i, NC)
        c = j if r % 2 == 0 else NC - 1 - j
        arr[r, c] = t

    # local search: swap tiles between slot groups to reduce padded edge tiles
    ceil_t = np.ceil(cnt / 128).astype(np.int64)
    costs = np.array([ceil_t[arr[s]].max(axis=0).sum() for s in range(SLOTS)])
    rng = np.random.default_rng(0)
    for _ in range(opt_iters):
        s1, s2 = rng.integers(0, SLOTS, 2)
        if s1 == s2:
            continue
        i1, i2 = rng.integers(0, NC, 2)
        a, b = arr[s1, i1], arr[s2, i2]
        arr[s1, i1], arr[s2, i2] = b, a
        c1 = ceil_t[arr[s1]].max(axis=0).sum()
        c2 = ceil_t[arr[s2]].max(axis=0).sum()
        if c1 + c2 <= costs[s1] + costs[s2]:
            costs[s1], costs[s2] = c1, c2
        else:
            arr[s1, i1], arr[s2, i2] = a, b
    perm = np.ascontiguousarray(arr.T)  # [NC, SLOTS]

    core_of_tile = np.empty(NTILES, dtype=np.int64)
    slot_of_tile = np.empty(NTILES, dtype=np.int64)
    for c in range(NC):
        core_of_tile[perm[c]] = c
        slot_of_tile[perm[c]] = np.arange(SLOTS)

    # uniform edge-tile counts: T[s][b] = max over cores
    C = cnt[perm]  # [NC, SLOTS, NB]
    T = np.ceil(C.max(axis=0) / 128).astype(np.int64)  # [SLOTS, NB]

    # flat layout in (group, bank, slot) order: edge segments and et columns
    seg_edge_off = np.zeros((SLOTS, NB), dtype=np.int64)  # offset in padded edge stream
    et_col = np.zeros((SLOTS, NB), dtype=np.int64)  # first et column index
    grp_gather_off = np.zeros((NGROUPS, NB), dtype=np.int64)  # edge offset of each gather
    grp_gather_sz = np.zeros((NGROUPS, NB), dtype=np.int64)  # edges per gather
    off_e = 0
    off_c = 0
    for g in range(NGROUPS):
        ss = range(g * G, min((g + 1) * G, SLOTS))
        for b in range(NB):
            grp_gather_off[g, b] = off_e
            for s in ss:
                seg_edge_off[s, b] = off_e
                et_col[s, b] = off_c
                off_e += T[s, b] * 128
                off_c += T[s, b]
            grp_gather_sz[g, b] = off_e - grp_gather_off[g, b]
    epad = off_e
    et_total = off_c
    return dict(
        tile_id=tile_id, bank=bank, perm=perm, core_of_tile=core_of_tile,
        slot_of_tile=slot_of_tile, T=T, seg_edge_off=seg_edge_off,
        et_col=et_col, grp_gather_off=grp_gather_off, grp_gather_sz=grp_gather_sz,
        epad=int(epad), et_total=int(et_total),
    )


def _pack_host_data(features, src, dst, W, b, gamma, beta, skip_W, skip_b, plan):
    """Build shared (replicated) and per-core input arrays."""
    T = plan["T"]
    epad, et_total = plan["epad"], plan["et_total"]

    deg_out = np.bincount(src, minlength=N).astype(np.float32)
    deg_in = np.bincount(dst, minlength=N).astype(np.float32)
    norm_out = 1.0 / np.sqrt(np.maximum(deg_out, 1.0))
    norm_in = 1.0 / np.sqrt(np.maximum(deg_in, 1.0))
    normprod = (norm_out[src] * norm_in[dst]).astype(np.float32)

    # order edges by (core, group, bank, slot, src)
    core_e = plan["core_of_tile"][plan["tile_id"]]
    slot_e = plan["slot_of_tile"][plan["tile_id"]]
    group_e = slot_e // G
    order = np.lexsort((src, slot_e, plan["bank"], group_e, core_e))
    src_o = src[order]
    dst_o = dst[order]
    bank_o = plan["bank"][order]
    core_o = core_e[order]
    slot_o = slot_e[order]
    np_o = normprod[order]

    # rank within each (core, slot, bank) run
    E = len(src_o)
    key_change = np.ones(E, dtype=bool)
    key_change[1:] = (
        (core_o[1:] != core_o[:-1]) | (slot_o[1:] != slot_o[:-1]) | (bank_o[1:] != bank_o[:-1])
    )
    run_start = np.maximum.accumulate(np.where(key_change, np.arange(E), 0))
    rank = np.arange(E) - run_start

    pos = plan["seg_edge_off"][slot_o, bank_o] + rank  # position in padded stream
    assert (rank < T[slot_o, bank_o] * 128).all()

    idx_pad = np.zeros((NC, epad), dtype=np.int16)
    slot_pad = np.zeros((NC, epad), dtype=np.float32)
    norm_pad = np.zeros((NC, epad), dtype=np.float32)
    idx_pad[core_o, pos] = (src_o - bank_o * BS).astype(np.int16)
    slot_pad[core_o, pos] = (dst_o - plan["perm"][core_o, slot_o] * TD).astype(np.float32)
    norm_pad[core_o, pos] = np_o

    # wrapped int16 idx layout: per 16-edge column, replicated over 8x16 partitions
    idx_w = np.ascontiguousarray(
        np.tile(idx_pad.reshape(NC, epad // 16, 16).transpose(0, 2, 1), (1, 8, 1))
    )  # [NC, 128, epad/16]
    # slot/norm layout: edge i -> partition i%128, col i//128
    slot_w = np.ascontiguousarray(slot_pad.reshape(NC, et_total, 128).transpose(0, 2, 1))
    norm_w = np.ascontiguousarray(norm_pad.reshape(NC, et_total, 128).transpose(0, 2, 1))

    # fp16 feature banks (zero-padded to NP rows)
    fpad16 = np.zeros((NP, F), dtype=np.float16)
    fpad16[:N] = features.astype(np.float16)
    fbanks = [np.ascontiguousarray(fpad16[k * BS:(k + 1) * BS]) for k in range(NB)]

    # per-core transposed skip features in slot order (fp16 like the gather path)
    featT = np.empty((NC, F, SLOTS * TD), dtype=np.float16)
    for c in range(NC):
        rows = (plan["perm"][c][:, None] * TD + np.arange(TD)[None, :]).reshape(-1)
        featT[c] = fpad16[rows].T

    shared = dict(
        iota=np.ascontiguousarray(np.broadcast_to(np.arange(TD, dtype=np.float16), (128, TD))),
        Wh=b_cast16(W), brow=b.astype(np.float16).reshape(1, HID),
        skipW=skip_W.astype(np.float16), skipbrow=skip_b.astype(np.float32).reshape(1, HID),
        ones16=np.ones((1, 128), dtype=np.float16),
        ones32=np.ones((1, 128), dtype=np.float32),
        gammab=np.ascontiguousarray(np.broadcast_to(gamma.astype(np.float32), (128, HID))),
        betab=np.ascontiguousarray(np.broadcast_to(beta.astype(np.float32), (128, HID))),
    )
    for k in range(NB):
        shared[f"fb{k}"] = fbanks[k]

    per_core = []
    for c in range(NC):
        per_core.append(dict(
            idx=idx_w[c], slotv=slot_w[c], normv=norm_w[c], featT=featT[c],
        ))
    return shared, per_core


def b_cast16(W):
    return W.astype(np.float16)


# ---------------- bass program ----------------

def build_program(plan, trivial_affine, trivial_b=False, trivial_skipb=False, debug=False):
    """One SPMD program; structure depends only on plan['T'] (+ affine/bias triviality)."""
    T = plan["T"]
    epad, et_total = plan["epad"], plan["et_total"]

    nc = bacc.Bacc("TRN2", target_bir_lowering=False, debug=debug, num_swdge_queues=4)

    d_fb = [nc.dram_tensor(f"fb{k}", [BS, F], f16, kind="ExternalInput") for k in range(NB)]
    d_idx = nc.dram_tensor("idx", [128, epad // 16], i16, kind="ExternalInput")
    d_slot = nc.dram_tensor("slotv", [128, et_total], f32, kind="ExternalInput")
    d_norm = nc.dram_tensor("normv", [128, et_total], f32, kind="ExternalInput")
    d_featT = nc.dram_tensor("featT", [F, SLOTS * TD], f16, kind="ExternalInput")
    d_iota = nc.dram_tensor("iota", [128, TD], f16, kind="ExternalInput")
    d_W = nc.dram_tensor("Wh", [F, HID], f16, kind="ExternalInput")
    d_brow = nc.dram_tensor("brow", [1, HID], f16, kind="ExternalInput")
    d_skipW = nc.dram_tensor("skipW", [F, HID], f16, kind="ExternalInput")
    d_skipbrow = nc.dram_tensor("skipbrow", [1, HID], f32, kind="ExternalInput")
    d_ones16 = nc.dram_tensor("ones16", [1, 128], f16, kind="ExternalInput")
    d_ones32 = nc.dram_tensor("ones32", [1, 128], f32, kind="ExternalInput")
    d_gammab = nc.dram_tensor("gammab", [128, HID], f32, kind="ExternalInput")
    d_betab = nc.dram_tensor("betab", [128, HID], f32, kind="ExternalInput")
    d_out = nc.dram_tensor("out", [SLOTS * TD, HID], f32, kind="ExternalOutput")
    out_v = d_out[:].rearrange("(s p) h -> s p h", p=TD)  # [SLOTS, 128, HID]

    import itertools
    qrr = itertools.cycle(range(4))  # round-robin SWDGE queue for gather chunks

    with tile.TileContext(nc) as tc:
        with (
            tc.tile_pool(name="const", bufs=1) as const,
            tc.tile_pool(name="meta", bufs=2) as meta,
            tc.tile_pool(name="hpool", bufs=2) as hpool,
            tc.tile_pool(name="spool", bufs=4) as spool,
            tc.tile_pool(name="stats", bufs=4) as stats,
            tc.tile_pool(name="opool", bufs=2) as opool,
            tc.tile_pool(name="psA", bufs=2, space="PSUM") as psA,
            tc.tile_pool(name="psG", bufs=2, space="PSUM") as psG,
            tc.tile_pool(name="psS", bufs=2, space="PSUM") as psS,
        ):
            t_iota = const.tile([128, TD], f16)
            nc.sync.dma_start(t_iota[:], d_iota[:])
            t_W = const.tile([F, HID], f16)
            nc.sync.dma_start(t_W[:], d_W[:])
            t_brow = const.tile([1, HID], f16)
            nc.sync.dma_start(t_brow[:], d_brow[:])
            t_skipW = const.tile([F, HID], f16)
            nc.sync.dma_start(t_skipW[:], d_skipW[:])
            if not trivial_skipb:
                t_skipbrow = const.tile([1, HID], f32)
                nc.sync.dma_start(t_skipbrow[:], d_skipbrow[:])
            t_ones16 = const.tile([1, 128], f16)
            nc.sync.dma_start(t_ones16[:], d_ones16[:])
            t_ones32 = const.tile([1, 128], f32)
            nc.sync.dma_start(t_ones32[:], d_ones32[:])
            if not trivial_affine:
                t_gammab = const.tile([128, HID], f32)
                nc.sync.dma_start(t_gammab[:], d_gammab[:])
                t_betab = const.tile([128, HID], f32)
                nc.sync.dma_start(t_betab[:], d_betab[:])
            t_eps = const.tile([128, 1], f32)
            nc.vector.memset(t_eps[:], EPS)

            for g in range(NGROUPS):
                s_lo = g * G
                s_hi = min(s_lo + G, SLOTS)
                ns = s_hi - s_lo
                gt = [int(plan["grp_gather_sz"][g, b]) for b in range(NB)]
                goff = [int(plan["grp_gather_off"][g, b]) for b in range(NB)]
                c_lo = int(plan["et_col"][s_lo, 0])
                c_hi = c_lo + sum(gt) // 128

                # group metadata loads
                t_idx = meta.tile([128, sum(gt) // 16], i16, tag="idx")
                nc.sync.dma_start(t_idx[:], d_idx[:, goff[0] // 16: goff[0] // 16 + sum(gt) // 16])
                t_slot = meta.tile([128, c_hi - c_lo], f32, tag="slot")
                nc.sync.dma_start(t_slot[:], d_slot[:, c_lo:c_hi])
                t_norm = meta.tile([128, c_hi - c_lo], f32, tag="norm")
                nc.sync.dma_start(t_norm[:], d_norm[:, c_lo:c_hi])
                t_featT = meta.tile([F, ns * TD], f16, tag="featT")
                nc.sync.dma_start(t_featT[:], d_featT[:, s_lo * TD: s_hi * TD])

                # gathers (per bank, chunked to <=1024 idxs per instruction --
                # the gather ucode's Q7 scratch caps num_idxs; 4 SWDGE queues
                # let 4 chunk desc-gens run on distinct Q7 core pairs)
                t_H = []
                for bk in range(NB):
                    if gt[bk] == 0:
                        t_H.append(None)
                        continue
                    th = hpool.tile([128, gt[bk] // 128, F], f16, tag=f"H{bk}")
                    for ch in range(0, gt[bk], GCH):
                        sz = min(GCH, gt[bk] - ch)
                        off16 = (goff[bk] - goff[0] + ch) // 16
                        nc.gpsimd.dma_gather(
                            th[:, ch // 128: (ch + sz) // 128, :], d_fb[bk][:],
                            t_idx[:, off16: off16 + sz // 16],
                            sz, sz, F, queue_num=next(qrr),
                        )
                    t_H.append(th)

                t_out = opool.tile([128, ns, HID], f32, tag="out")

                for s in range(s_lo, s_hi):
                    n_et = int(T[s].sum())
                    # ---- aggregation ----
                    if n_et > 0:
                        t_aggT_ps = psA.tile([F, TD], f32, tag="aggT")
                        k = 0
                        for bk in range(NB):
                            h_base = (int(plan["seg_edge_off"][s, bk]) - goff[bk]) // 128
                            c_base = int(plan["et_col"][s, bk]) - c_lo
                            for e in range(int(T[s, bk])):
                                t_S = spool.tile([128, TD], f16, tag="S")
                                nc.vector.tensor_scalar(
                                    out=t_S[:], in0=t_iota[:],
                                    scalar1=t_slot[:, c_base + e: c_base + e + 1],
                                    scalar2=t_norm[:, c_base + e: c_base + e + 1],
                                    op0=mybir.AluOpType.is_equal,
                                    op1=mybir.AluOpType.mult,
                                )
                                nc.tensor.matmul(
                                    out=t_aggT_ps[:],
                                    lhsT=t_H[bk][:, h_base + e, :],
                                    rhs=t_S[:],
                                    start=(k == 0), stop=(k == n_et - 1),
                                )
                                k += 1
                        t_aggT = spool.tile([F, TD], f16, tag="aggT_sb")
                        nc.scalar.activation(
                            out=t_aggT[:], in_=t_aggT_ps[:],
                            func=mybir.ActivationFunctionType.Copy,
                        )

                    # ---- gcn = agg @ W + b ----
                    t_gcn_ps = psG.tile([TD, HID], f32, tag="gcn")
                    need_brow = (not trivial_b) or n_et == 0
                    if need_brow:
                        nc.tensor.matmul(
                            out=t_gcn_ps[:], lhsT=t_ones16[:], rhs=t_brow[:],
                            start=True, stop=(n_et == 0),
                        )
                    if n_et > 0:
                        nc.tensor.matmul(
                            out=t_gcn_ps[:], lhsT=t_aggT[:], rhs=t_W[:],
                            start=not need_brow, stop=True,
                        )

                    # ---- skip = feat @ skip_W + skip_b ----
                    t_skip_ps = psS.tile([TD, HID], f32, tag="skip")
                    if not trivial_skipb:
                        nc.tensor.matmul(
                            out=t_skip_ps[:], lhsT=t_ones32[:], rhs=t_skipbrow[:],
                            start=True, stop=False,
                        )
                    nc.tensor.matmul(
                        out=t_skip_ps[:], lhsT=t_featT[:, (s - s_lo) * TD:(s - s_lo + 1) * TD],
                        rhs=t_skipW[:], start=trivial_skipb, stop=True,
                    )

                    # ---- layernorm + relu + skip add ----
                    t_stats = stats.tile([TD, 6], f32, tag="bn")
                    nc.vector.bn_stats(out=t_stats[:], in_=t_gcn_ps[:])
                    t_mv = stats.tile([TD, 2], f32, tag="mv")
                    nc.vector.bn_aggr(out=t_mv[:], in_=t_stats[:])
                    t_std = stats.tile([TD, 1], f32, tag="std")
                    nc.scalar.activation(
                        out=t_std[:], in_=t_mv[:, 1:2],
                        func=mybir.ActivationFunctionType.Sqrt, bias=t_eps[:],
                    )
                    t_rstd = stats.tile([TD, 1], f32, tag="rstd")
                    nc.vector.reciprocal(out=t_rstd[:], in_=t_std[:])
                    t_y = spool.tile([TD, HID], f32, tag="y")
                    nc.vector.tensor_scalar(
                        out=t_y[:], in0=t_gcn_ps[:],
                        scalar1=t_mv[:, 0:1], scalar2=t_rstd[:],
                        op0=mybir.AluOpType.subtract, op1=mybir.AluOpType.mult,
                    )
                    if not trivial_affine:
                        nc.vector.tensor_tensor(
                            out=t_y[:], in0=t_y[:], in1=t_gammab[:], op=mybir.AluOpType.mult
                        )
                        nc.vector.tensor_tensor(
                            out=t_y[:], in0=t_y[:], in1=t_betab[:], op=mybir.AluOpType.add
                        )
                    t_r = spool.tile([TD, HID], f32, tag="r")
                    nc.scalar.activation(
                        out=t_r[:], in_=t_y[:], func=mybir.ActivationFunctionType.Relu
                    )
                    nc.vector.tensor_tensor(
                        out=t_out[:, s - s_lo, :], in0=t_r[:], in1=t_skip_ps[:],
                        op=mybir.AluOpType.add,
                    )

                nc.sync.dma_start(
                    out_v[s_lo:s_hi].rearrange("s p h -> p s h"), t_out[:, :ns, :]
                )

    nc.compile()
    return nc


# ---------------- public entry ----------------

_CACHE = {}
_LAST = {}


def kernel(features, src, dst, W, b, gamma, beta, skip_W, skip_b):
    features = np.asarray(features, dtype=np.float32)
    src = np.asarray(src).astype(np.int64)
    dst = np.asarray(dst).astype(np.int64)
    W = np.asarray(W, dtype=np.float32)
    b = np.asarray(b, dtype=np.float32)
    gamma = np.asarray(gamma, dtype=np.float32)
    beta = np.asarray(beta, dtype=np.float32)
    skip_W = np.asarray(skip_W, dtype=np.float32)
    skip_b = np.asarray(skip_b, dtype=np.float32)

    plan = _plan(src, dst)
    shared, per_core = _pack_host_data(
        features, src, dst, W, b, gamma, beta, skip_W, skip_b, plan
    )
    trivial_affine = bool(np.all(gamma == 1.0) and np.all(beta == 0.0))
    trivial_b = bool(np.all(b == 0.0))
    trivial_skipb = bool(np.all(skip_b == 0.0))

    key = (plan["T"].tobytes(), trivial_affine, trivial_b, trivial_skipb)
    if key not in _CACHE:
        _CACHE[key] = build_program(plan, trivial_affine, trivial_b, trivial_skipb)
    nc = _CACHE[key]

    from concourse.bass_utils import run_bass_kernel_spmd

    in_maps = [{**shared, **pc} for pc in per_core]
    _LAST.update(plan=plan, nc=nc, in_maps=in_maps)
    res = run_bass_kernel_spmd(nc, in_maps, core_ids=list(range(NC)))

    out_full = np.empty((NP, HID), dtype=np.float32)
    for c in range(NC):
        oc = res.results[c]["out"].reshape(SLOTS, TD, HID)
        out_full[plan["perm"][c][:, None] * TD + np.arange(TD)[None, :]] = oc
    return out_full[:N]



# revision 10
# speedup vs baseline: 1.0946x; 1.0946x over previous
"""GCN block (GraphConv + LayerNorm + ReLU + skip projection) on 8 Trainium2 cores.

Strategy (dst-node sharding, per spec sharding_hint):
- 100000 dst nodes -> 784 tiles of 128 dsts (padded to 100352); tiles snake-dealt
  to 8 cores by edge count so every core runs an identical (SPMD) program.
- Edges routed to the core owning their dst tile. Per (tile, src-bank) edge lists
  are padded to multiples of 128; the per-slot/bank edge-tile counts are made
  uniform across cores (max), so one NEFF serves all cores.
- Normalization folding: the gathered feature bank is pre-scaled by norm_src on
  the host, and norm_dst is dropped entirely — LayerNorm is invariant to a
  per-row scale, and the b!=0 case is fixed exactly by adding (1/norm_dst) (x) b
  via an outer-product matmul (LN(nd*(agg@W + b/nd)) == LN of the true gcn row).
  The S matrix is then a pure 0/1 selection mask: S[e, d] = (slot_e == d),
  built for a whole slot in ONE DVE tensor_tensor(is_equal) using stride-0
  broadcast APs (padded edges carry slot=-1 so they never match).
- Aggregation agg^T = H^T S per 128-edge tile on TensorE; H = gathered fp16
  pre-scaled src rows (dma_gather, int16 idxs => 4 feature banks of 25088 rows,
  one big gather per (group, bank) on its own SWDGE queue).
- gcn = agg @ W (+ ndinv (x) b if b!=0); LayerNorm via bn_stats/bn_aggr; skip =
  features @ skip_W (+ skip_b); relu + add; fp16 output DMA per 8-slot group.
"""

import sys

sys.path.insert(0, "/opt/trn_rl_repo")

import numpy as np

import concourse.bass as bass  # noqa: F401
import concourse.tile as tile
from concourse import bacc, mybir

# ---------------- problem constants (hardcoded per spec) ----------------
N = 100000
F = 128
HID = 256
NC = 8
TD = 128  # dsts per tile
EPS = 1e-5
NTILES = 784  # ceil(100000/128)=782, padded to a multiple of NC
NP = NTILES * TD  # 100352 padded node space
NB = 4  # src banks (dma_gather idxs are int16)
BS = NP // NB  # 25088 rows per bank
SLOTS = NTILES // NC  # 98 per core
G = 8  # slots per gather group
NGROUPS = (SLOTS + G - 1) // G  # 13
GCH = 1024  # max idxs per dma_gather instruction (HW-capped; 2048+ crashes device)

f16 = mybir.dt.float16
f32 = mybir.dt.float32
i16 = mybir.dt.int16


# ---------------- host-side graph preprocessing ----------------

def _plan(src, dst, opt_iters=600_000):
    """Compute the SPMD-uniform structure: tile->core deal, per (slot, bank)
    edge-tile counts T[s][b], and the flat segment layout. Edge segments are
    laid out (group, bank, slot)-major (matching the per-bank gathers); the
    et (edge-tile) columns are laid out (group, slot, bank)-major so each
    slot's columns are contiguous for the one-shot S build.

    Tiles are grouped into slots of NC so that the per-slot/bank max (which all
    cores pad to) is small: snake-deal by total count, then local-search swaps
    minimizing sum_s,b max_c ceil(cnt/128). Deterministic (fixed iteration
    count) so repeated runs produce identical programs and hit the NEFF cache."""
    tile_id = dst // TD
    bank = src // BS

    cnt = np.zeros((NTILES, NB), dtype=np.int64)
    np.add.at(cnt, (tile_id, bank), 1)
    tot = cnt.sum(1)

    # snake-deal tiles (desc by edge count) to slot groups
    order = np.argsort(-tot, kind="stable")
    arr = np.empty((SLOTS, NC), dtype=np.int64)
    for i, t in enumerate(order):
        r, j = divmod(i, NC)
        c = j if r % 2 == 0 else NC - 1 - j
        arr[r, c] = t

    # local search: swap tiles between slot groups to reduce padded edge tiles
    ceil_t = np.ceil(cnt / 128).astype(np.int64)
    costs = np.array([ceil_t[arr[s]].max(axis=0).sum() for s in range(SLOTS)])
    rng = np.random.default_rng(0)
    for _ in range(opt_iters):
        s1, s2 = rng.integers(0, SLOTS, 2)
        if s1 == s2:
            continue
        i1, i2 = rng.integers(0, NC, 2)
        a, b = arr[s1, i1], arr[s2, i2]
        arr[s1, i1], arr[s2, i2] = b, a
        c1 = ceil_t[arr[s1]].max(axis=0).sum()
        c2 = ceil_t[arr[s2]].max(axis=0).sum()
        if c1 + c2 <= costs[s1] + costs[s2]:
            costs[s1], costs[s2] = c1, c2
        else:
            arr[s1, i1], arr[s2, i2] = a, b
    perm = np.ascontiguousarray(arr.T)  # [NC, SLOTS]

    core_of_tile = np.empty(NTILES, dtype=np.int64)
    slot_of_tile = np.empty(NTILES, dtype=np.int64)
    for c in range(NC):
        core_of_tile[perm[c]] = c
        slot_of_tile[perm[c]] = np.arange(SLOTS)

    # uniform edge-tile counts: T[s][b] = max over cores
    C = cnt[perm]  # [NC, SLOTS, NB]
    T = np.ceil(C.max(axis=0) / 128).astype(np.int64)  # [SLOTS, NB]

    # edge segments in (group, bank, slot) order; et columns in
    # (group, slot, bank) order (slot-contiguous).
    seg_edge_off = np.zeros((SLOTS, NB), dtype=np.int64)  # offset in padded edge stream
    et_col = np.zeros((SLOTS, NB), dtype=np.int64)  # first et column index
    grp_gather_off = np.zeros((NGROUPS, NB), dtype=np.int64)  # edge offset of each gather
    grp_gather_sz = np.zeros((NGROUPS, NB), dtype=np.int64)  # edges per gather
    off_e = 0
    off_c = 0
    for g in range(NGROUPS):
        ss = range(g * G, min((g + 1) * G, SLOTS))
        for b in range(NB):
            grp_gather_off[g, b] = off_e
            for s in ss:
                seg_edge_off[s, b] = off_e
                off_e += T[s, b] * 128
            grp_gather_sz[g, b] = off_e - grp_gather_off[g, b]
        for s in ss:
            for b in range(NB):
                et_col[s, b] = off_c
                off_c += T[s, b]
    epad = off_e
    et_total = off_c
    return dict(
        tile_id=tile_id, bank=bank, perm=perm, core_of_tile=core_of_tile,
        slot_of_tile=slot_of_tile, T=T, seg_edge_off=seg_edge_off,
        et_col=et_col, grp_gather_off=grp_gather_off, grp_gather_sz=grp_gather_sz,
        epad=int(epad), et_total=int(et_total),
    )


def _pack_host_data(features, src, dst, W, b, gamma, beta, skip_W, skip_b, plan):
    """Build shared (replicated) and per-core input arrays."""
    T = plan["T"]
    epad, et_total = plan["epad"], plan["et_total"]

    deg_out = np.bincount(src, minlength=N).astype(np.float32)
    deg_in = np.bincount(dst, minlength=N).astype(np.float32)
    norm_out = 1.0 / np.sqrt(np.maximum(deg_out, 1.0))
    ndinv = np.sqrt(np.maximum(deg_in, 1.0))  # 1/norm_dst, for the b!=0 path

    # order edges by (core, group, bank, slot, src)
    core_e = plan["core_of_tile"][plan["tile_id"]]
    slot_e = plan["slot_of_tile"][plan["tile_id"]]
    group_e = slot_e // G
    order = np.lexsort((src, slot_e, plan["bank"], group_e, core_e))
    src_o = src[order]
    dst_o = dst[order]
    bank_o = plan["bank"][order]
    core_o = core_e[order]
    slot_o = slot_e[order]

    # rank within each (core, slot, bank) run
    E = len(src_o)
    key_change = np.ones(E, dtype=bool)
    key_change[1:] = (
        (core_o[1:] != core_o[:-1]) | (slot_o[1:] != slot_o[:-1]) | (bank_o[1:] != bank_o[:-1])
    )
    run_start = np.maximum.accumulate(np.where(key_change, np.arange(E), 0))
    rank = np.arange(E) - run_start

    pos = plan["seg_edge_off"][slot_o, bank_o] + rank  # position in padded stream
    assert (rank < T[slot_o, bank_o] * 128).all()

    idx_pad = np.zeros((NC, epad), dtype=np.int16)
    # padded (unused) edges carry slot=-1 so the 0/1 selection mask zeroes them
    slot_pad = np.full((NC, epad), -1.0, dtype=np.float16)
    idx_pad[core_o, pos] = (src_o - bank_o * BS).astype(np.int16)
    slot_pad[core_o, pos] = (dst_o - plan["perm"][core_o, slot_o] * TD).astype(np.float16)

    # wrapped int16 idx layout: per 16-edge column, replicated over 8x16 partitions
    idx_w = np.ascontiguousarray(
        np.tile(idx_pad.reshape(NC, epad // 16, 16).transpose(0, 2, 1), (1, 8, 1))
    )  # [NC, 128, epad/16]
    # slot layout: edge i -> partition i%128, et column; but the et-column order
    # differs from the edge-stream order, so map through seg offsets.
    slot_w = np.empty((NC, 128, et_total), dtype=np.float16)
    slot_seg = slot_pad.reshape(NC, epad // 128, 128).transpose(0, 2, 1)  # by edge col
    for s in range(SLOTS):
        for bk in range(NB):
            n = int(T[s, bk])
            e0 = int(plan["seg_edge_off"][s, bk]) // 128
            c0 = int(plan["et_col"][s, bk])
            slot_w[:, :, c0:c0 + n] = slot_seg[:, :, e0:e0 + n]

    # fp16 feature banks pre-scaled by norm_src (zero-padded to NP rows)
    fpad16 = np.zeros((NP, F), dtype=np.float16)
    fpad16[:N] = (features * norm_out[:, None]).astype(np.float16)
    fbanks = [np.ascontiguousarray(fpad16[k * BS:(k + 1) * BS]) for k in range(NB)]

    # per-core transposed skip features in slot order (unscaled), fp16
    rawpad16 = np.zeros((NP, F), dtype=np.float16)
    rawpad16[:N] = features.astype(np.float16)
    featT = np.empty((NC, F, SLOTS * TD), dtype=np.float16)
    ndinv_pc = np.ones((NC, 1, SLOTS * TD), dtype=np.float16)
    ndinv_pad = np.ones(NP, dtype=np.float32)
    ndinv_pad[:N] = ndinv
    for c in range(NC):
        rows = (plan["perm"][c][:, None] * TD + np.arange(TD)[None, :]).reshape(-1)
        featT[c] = rawpad16[rows].T
        ndinv_pc[c, 0] = ndinv_pad[rows].astype(np.float16)

    shared = dict(
        iota=np.ascontiguousarray(np.broadcast_to(np.arange(TD, dtype=np.float16), (128, TD))),
        Wh=W.astype(np.float16), brow=b.astype(np.float16).reshape(1, HID),
        skipW=skip_W.astype(np.float16), skipbrow=skip_b.astype(np.float32).reshape(1, HID),
        ones16=np.ones((1, 128), dtype=np.float16),
        ones32=np.ones((1, 128), dtype=np.float32),
        gammab=np.ascontiguousarray(np.broadcast_to(gamma.astype(np.float32), (128, HID))),
        betab=np.ascontiguousarray(np.broadcast_to(beta.astype(np.float32), (128, HID))),
    )
    for k in range(NB):
        shared[f"fb{k}"] = fbanks[k]

    per_core = []
    for c in range(NC):
        per_core.append(dict(
            idx=idx_w[c], slotv=slot_w[c], featT=featT[c], ndinv=ndinv_pc[c],
        ))
    return shared, per_core


# ---------------- bass program ----------------

def build_program(plan, trivial_affine, trivial_b=False, trivial_skipb=False, debug=False):
    """One SPMD program; structure depends only on plan['T'] (+ affine/bias triviality)."""
    T = plan["T"]
    epad, et_total = plan["epad"], plan["et_total"]

    nc = bacc.Bacc("TRN2", target_bir_lowering=False, debug=debug, num_swdge_queues=4)

    d_fb = [nc.dram_tensor(f"fb{k}", [BS, F], f16, kind="ExternalInput") for k in range(NB)]
    d_idx = nc.dram_tensor("idx", [128, epad // 16], i16, kind="ExternalInput")
    d_slot = nc.dram_tensor("slotv", [128, et_total], f16, kind="ExternalInput")
    d_featT = nc.dram_tensor("featT", [F, SLOTS * TD], f16, kind="ExternalInput")
    d_ndinv = nc.dram_tensor("ndinv", [1, SLOTS * TD], f16, kind="ExternalInput")
    d_iota = nc.dram_tensor("iota", [128, TD], f16, kind="ExternalInput")
    d_W = nc.dram_tensor("Wh", [F, HID], f16, kind="ExternalInput")
    d_brow = nc.dram_tensor("brow", [1, HID], f16, kind="ExternalInput")
    d_skipW = nc.dram_tensor("skipW", [F, HID], f16, kind="ExternalInput")
    d_skipbrow = nc.dram_tensor("skipbrow", [1, HID], f32, kind="ExternalInput")
    d_ones16 = nc.dram_tensor("ones16", [1, 128], f16, kind="ExternalInput")
    d_ones32 = nc.dram_tensor("ones32", [1, 128], f32, kind="ExternalInput")
    d_gammab = nc.dram_tensor("gammab", [128, HID], f32, kind="ExternalInput")
    d_betab = nc.dram_tensor("betab", [128, HID], f32, kind="ExternalInput")
    d_out = nc.dram_tensor("out", [SLOTS * TD, HID], f16, kind="ExternalOutput")
    out_v = d_out[:].rearrange("(s p) h -> s p h", p=TD)  # [SLOTS, 128, HID]

    with tile.TileContext(nc) as tc:
        with (
            tc.tile_pool(name="const", bufs=1) as const,
            tc.tile_pool(name="meta", bufs=2) as meta,
            tc.tile_pool(name="hpool", bufs=2) as hpool,
            tc.tile_pool(name="spool", bufs=2) as spool,
            tc.tile_pool(name="ypool", bufs=4) as ypool,
            tc.tile_pool(name="stats", bufs=4) as stats,
            tc.tile_pool(name="opool", bufs=2) as opool,
            tc.tile_pool(name="psA", bufs=2, space="PSUM") as psA,
            tc.tile_pool(name="psG", bufs=2, space="PSUM") as psG,
            tc.tile_pool(name="psS", bufs=2, space="PSUM") as psS,
        ):
            t_iota = const.tile([128, TD], f16)
            nc.sync.dma_start(t_iota[:], d_iota[:])
            t_W = const.tile([F, HID], f16)
            nc.sync.dma_start(t_W[:], d_W[:])
            t_brow = const.tile([1, HID], f16)
            nc.sync.dma_start(t_brow[:], d_brow[:])
            t_skipW = const.tile([F, HID], f16)
            nc.sync.dma_start(t_skipW[:], d_skipW[:])
            t_ones16 = const.tile([1, 128], f16)
            nc.sync.dma_start(t_ones16[:], d_ones16[:])
            if not trivial_b:
                t_ndinv = const.tile([1, SLOTS * TD], f16)
                nc.sync.dma_start(t_ndinv[:], d_ndinv[:])
            if not trivial_skipb:
                t_skipbrow = const.tile([1, HID], f32)
                nc.sync.dma_start(t_skipbrow[:], d_skipbrow[:])
                t_ones32 = const.tile([1, 128], f32)
                nc.sync.dma_start(t_ones32[:], d_ones32[:])
            if not trivial_affine:
                t_gammab = const.tile([128, HID], f32)
                nc.sync.dma_start(t_gammab[:], d_gammab[:])
                t_betab = const.tile([128, HID], f32)
                nc.sync.dma_start(t_betab[:], d_betab[:])
            t_eps = const.tile([128, 1], f32)
            nc.vector.memset(t_eps[:], EPS)

            for g in range(NGROUPS):
                s_lo = g * G
                s_hi = min(s_lo + G, SLOTS)
                ns = s_hi - s_lo
                gt = [int(plan["grp_gather_sz"][g, b]) for b in range(NB)]
                goff = [int(plan["grp_gather_off"][g, b]) for b in range(NB)]
                c_lo = int(plan["et_col"][s_lo, 0])
                c_hi = c_lo + sum(gt) // 128

                # group metadata loads
                t_idx = meta.tile([128, sum(gt) // 16], i16, tag="idx")
                nc.sync.dma_start(t_idx[:], d_idx[:, goff[0] // 16: goff[0] // 16 + sum(gt) // 16])
                t_slot = meta.tile([128, c_hi - c_lo], f16, tag="slot")
                nc.sync.dma_start(t_slot[:], d_slot[:, c_lo:c_hi])
                t_featT = meta.tile([F, ns * TD], f16, tag="featT")
                nc.sync.dma_start(t_featT[:], d_featT[:, s_lo * TD: s_hi * TD])

                # one big gather per (group, bank), each bank on its own SWDGE
                # queue so the 4 rings' DMAs overlap
                t_H = []
                for bk in range(NB):
                    if gt[bk] == 0:
                        t_H.append(None)
                        continue
                    th = hpool.tile([128, gt[bk] // 128, F], f16, tag=f"H{bk}")
                    for ch in range(0, gt[bk], GCH):
                        sz = min(GCH, gt[bk] - ch)
                        off16 = (goff[bk] - goff[0] + ch) // 16
                        nc.gpsimd.dma_gather(
                            th[:, ch // 128: (ch + sz) // 128, :], d_fb[bk][:],
                            t_idx[:, off16: off16 + sz // 16],
                            sz, sz, F, queue_num=bk,
                        )
                    t_H.append(th)

                t_out = opool.tile([128, ns, HID], f16, tag="out")

                for s in range(s_lo, s_hi):
                    n_et = int(T[s].sum())
                    # ---- selection masks for the whole slot in one DVE op ----
                    if n_et > 0:
                        c0 = int(plan["et_col"][s, 0]) - c_lo
                        t_S = spool.tile([128, n_et, TD], f16, tag="S")
                        nc.vector.tensor_tensor(
                            out=t_S[:],
                            in0=t_iota[:].unsqueeze(1).broadcast_to([128, n_et, TD]),
                            in1=t_slot[:, c0:c0 + n_et].unsqueeze(2).broadcast_to([128, n_et, TD]),
                            op=mybir.AluOpType.is_equal,
                        )
                        # ---- aggregation ----
                        t_aggT_ps = psA.tile([F, TD], f32, tag="aggT")
                        k = 0
                        for bk in range(NB):
                            h_base = (int(plan["seg_edge_off"][s, bk]) - goff[bk]) // 128
                            for e in range(int(T[s, bk])):
                                nc.tensor.matmul(
                                    out=t_aggT_ps[:],
                                    lhsT=t_H[bk][:, h_base + e, :],
                                    rhs=t_S[:, k, :],
                                    start=(k == 0), stop=(k == n_et - 1),
                                )
                                k += 1
                        t_aggT = ypool.tile([F, TD], f16, tag="aggT_sb")
                        nc.scalar.activation(
                            out=t_aggT[:], in_=t_aggT_ps[:],
                            func=mybir.ActivationFunctionType.Copy,
                        )

                    # ---- gcn = agg @ W (+ ndinv (x) b) ----
                    t_gcn_ps = psG.tile([TD, HID], f32, tag="gcn")
                    need_brow = (not trivial_b) or n_et == 0
                    if need_brow:
                        lhs_b = (
                            t_ndinv[:, s * TD:(s + 1) * TD] if not trivial_b
                            else t_ones16[:]
                        )
                        nc.tensor.matmul(
                            out=t_gcn_ps[:], lhsT=lhs_b, rhs=t_brow[:],
                            start=True, stop=(n_et == 0),
                        )
                    if n_et > 0:
                        nc.tensor.matmul(
                            out=t_gcn_ps[:], lhsT=t_aggT[:], rhs=t_W[:],
                            start=not need_brow, stop=True,
                        )

                    # ---- skip = feat @ skip_W + skip_b ----
                    t_skip_ps = psS.tile([TD, HID], f32, tag="skip")
                    if not trivial_skipb:
                        nc.tensor.matmul(
                            out=t_skip_ps[:], lhsT=t_ones32[:], rhs=t_skipbrow[:],
                            start=True, stop=False,
                        )
                    nc.tensor.matmul(
                        out=t_skip_ps[:], lhsT=t_featT[:, (s - s_lo) * TD:(s - s_lo + 1) * TD],
                        rhs=t_skipW[:], start=trivial_skipb, stop=True,
                    )

                    # ---- layernorm + relu + skip add ----
                    t_stats = stats.tile([TD, 6], f32, tag="bn")
                    nc.vector.bn_stats(out=t_stats[:], in_=t_gcn_ps[:])
                    t_mv = stats.tile([TD, 2], f32, tag="mv")
                    nc.vector.bn_aggr(out=t_mv[:], in_=t_stats[:])
                    t_std = stats.tile([TD, 1], f32, tag="std")
                    nc.scalar.activation(
                        out=t_std[:], in_=t_mv[:, 1:2],
                        func=mybir.ActivationFunctionType.Sqrt, bias=t_eps[:],
                    )
                    t_rstd = stats.tile([TD, 1], f32, tag="rstd")
                    nc.vector.reciprocal(out=t_rstd[:], in_=t_std[:])
                    t_y = ypool.tile([TD, HID], f32, tag="y")
                    nc.vector.tensor_scalar(
                        out=t_y[:], in0=t_gcn_ps[:],
                        scalar1=t_mv[:, 0:1], scalar2=t_rstd[:],
                        op0=mybir.AluOpType.subtract, op1=mybir.AluOpType.mult,
                    )
                    if not trivial_affine:
                        nc.vector.tensor_tensor(
                            out=t_y[:], in0=t_y[:], in1=t_gammab[:], op=mybir.AluOpType.mult
                        )
                        nc.vector.tensor_tensor(
                            out=t_y[:], in0=t_y[:], in1=t_betab[:], op=mybir.AluOpType.add
                        )
                    t_r = ypool.tile([TD, HID], f32, tag="r")
                    nc.scalar.activation(
                        out=t_r[:], in_=t_y[:], func=mybir.ActivationFunctionType.Relu
                    )
                    nc.vector.tensor_tensor(
                        out=t_out[:, s - s_lo, :], in0=t_r[:], in1=t_skip_ps[:],
                        op=mybir.AluOpType.add,
                    )

                nc.sync.dma_start(
                    out_v[s_lo:s_hi].rearrange("s p h -> p s h"), t_out[:, :ns, :]
                )

    nc.compile()
    return nc


# ---------------- public entry ----------------

_CACHE = {}
_LAST = {}


def kernel(features, src, dst, W, b, gamma, beta, skip_W, skip_b):
    features = np.asarray(features, dtype=np.float32)
    src = np.asarray(src).astype(np.int64)
    dst = np.asarray(dst).astype(np.int64)
    W = np.asarray(W, dtype=np.float32)
    b = np.asarray(b, dtype=np.float32)
    gamma = np.asarray(gamma, dtype=np.float32)
    beta = np.asarray(beta, dtype=np.float32)
    skip_W = np.asarray(skip_W, dtype=np.float32)
    skip_b = np.asarray(skip_b, dtype=np.float32)

    plan = _plan(src, dst)
    shared, per_core = _pack_host_data(
        features, src, dst, W, b, gamma, beta, skip_W, skip_b, plan
    )
    trivial_affine = bool(np.all(gamma == 1.0) and np.all(beta == 0.0))
    trivial_b = bool(np.all(b == 0.0))
    trivial_skipb = bool(np.all(skip_b == 0.0))

    key = (plan["T"].tobytes(), trivial_affine, trivial_b, trivial_skipb)
    if key not in _CACHE:
        _CACHE[key] = build_program(plan, trivial_affine, trivial_b, trivial_skipb)
    nc = _CACHE[key]

    from concourse.bass_utils import run_bass_kernel_spmd

    in_maps = [{**shared, **pc} for pc in per_core]
    _LAST.update(plan=plan, nc=nc, in_maps=in_maps)
    res = run_bass_kernel_spmd(nc, in_maps, core_ids=list(range(NC)))

    out_full = np.empty((NP, HID), dtype=np.float32)
    for c in range(NC):
        oc = res.results[c]["out"].astype(np.float32).reshape(SLOTS, TD, HID)
        out_full[plan["perm"][c][:, None] * TD + np.arange(TD)[None, :]] = oc
    return out_full[:N]


# revision 17
# speedup vs baseline: 1.3924x; 1.2721x over previous
"""GCN block (GraphConv + LayerNorm + ReLU + skip projection) on 8 Trainium2 cores.

Strategy (dst-node sharding, per spec sharding_hint):
- 100000 dst nodes -> 784 tiles of 128 dsts (padded to 100352); tiles snake-dealt
  to 8 cores by edge count so every core runs an identical (SPMD) program.
- Edges routed to the core owning their dst tile. Per (tile, src-bank) edge lists
  are padded to multiples of 128; the per-slot/bank edge-tile counts are made
  uniform across cores (max), so one NEFF serves all cores.
- Normalization folding: the gathered feature bank is pre-scaled by norm_src on
  the host, and norm_dst is dropped entirely — LayerNorm is invariant to a
  per-row scale, and the b!=0 case is fixed exactly by adding (1/norm_dst) (x) b
  via an outer-product matmul (LN(nd*(agg@W + b/nd)) == LN of the true gcn row).
  The S matrix is then a pure 0/1 selection mask: S[e, d] = (slot_e == d),
  built for a whole slot in ONE DVE tensor_tensor(is_equal) using stride-0
  broadcast APs (padded edges carry slot=-1 so they never match).
- Aggregation agg^T = H^T S per 128-edge tile on TensorE; H = gathered fp16
  pre-scaled src rows (dma_gather, int16 idxs => 4 feature banks of 25088 rows,
  one big gather per (group, bank) on its own SWDGE queue).
- gcn = agg @ W (+ ndinv (x) b if b!=0); LayerNorm via bn_stats/bn_aggr; skip =
  features @ skip_W (+ skip_b); relu + add; fp16 output DMA per 8-slot group.
"""

import sys

sys.path.insert(0, "/opt/trn_rl_repo")

import numpy as np

import concourse.bass as bass  # noqa: F401
import concourse.tile as tile
from concourse import bacc, mybir

# ---------------- problem constants (hardcoded per spec) ----------------
N = 100000
F = 128
HID = 256
NC = 8
TD = 128  # dsts per tile
EPS = 1e-5
NTILES = 784  # ceil(100000/128)=782, padded to a multiple of NC
NP = NTILES * TD  # 100352 padded node space
NB = 4  # src banks (dma_gather idxs are int16)
BS = NP // NB  # 25088 rows per bank
SLOTS = NTILES // NC  # 98 per core
G = 8  # slots per gather group
NGROUPS = (SLOTS + G - 1) // G  # 13
GCH = 1024  # max idxs per dma_gather instruction (hard HW cap: 1536+ crashes)

f16 = mybir.dt.float16
f32 = mybir.dt.float32
i16 = mybir.dt.int16


# ---------------- host-side graph preprocessing ----------------

def _plan(src, dst, opt_iters=600_000):
    """Compute the SPMD-uniform structure: tile->core deal, per (slot, bank)
    edge-tile counts T[s][b], and the flat segment layout. Edge segments are
    laid out (group, bank, slot)-major (matching the per-bank gathers); the
    et (edge-tile) columns are laid out (group, slot, bank)-major so each
    slot's columns are contiguous for the one-shot S build.

    Tiles are grouped into slots of NC so that the per-slot/bank max (which all
    cores pad to) is small: snake-deal by total count, then local-search swaps
    minimizing sum_s,b max_c ceil(cnt/128). Deterministic (fixed iteration
    count) so repeated runs produce identical programs and hit the NEFF cache."""
    tile_id = dst // TD
    bank = src // BS

    cnt = np.zeros((NTILES, NB), dtype=np.int64)
    np.add.at(cnt, (tile_id, bank), 1)
    tot = cnt.sum(1)

    # snake-deal tiles (desc by edge count) to slot groups
    order = np.argsort(-tot, kind="stable")
    arr = np.empty((SLOTS, NC), dtype=np.int64)
    for i, t in enumerate(order):
        r, j = divmod(i, NC)
        c = j if r % 2 == 0 else NC - 1 - j
        arr[r, c] = t

    # local search: swap tiles between slot groups to reduce padded edge tiles
    ceil_t = np.ceil(cnt / 128).astype(np.int64)
    costs = np.array([ceil_t[arr[s]].max(axis=0).sum() for s in range(SLOTS)])
    rng = np.random.default_rng(0)
    for _ in range(opt_iters):
        s1, s2 = rng.integers(0, SLOTS, 2)
        if s1 == s2:
            continue
        i1, i2 = rng.integers(0, NC, 2)
        a, b = arr[s1, i1], arr[s2, i2]
        arr[s1, i1], arr[s2, i2] = b, a
        c1 = ceil_t[arr[s1]].max(axis=0).sum()
        c2 = ceil_t[arr[s2]].max(axis=0).sum()
        if c1 + c2 <= costs[s1] + costs[s2]:
            costs[s1], costs[s2] = c1, c2
        else:
            arr[s1, i1], arr[s2, i2] = a, b
    perm = np.ascontiguousarray(arr.T)  # [NC, SLOTS]

    core_of_tile = np.empty(NTILES, dtype=np.int64)
    slot_of_tile = np.empty(NTILES, dtype=np.int64)
    for c in range(NC):
        core_of_tile[perm[c]] = c
        slot_of_tile[perm[c]] = np.arange(SLOTS)

    # uniform edge-tile counts: T[s][b] = max over cores
    C = cnt[perm]  # [NC, SLOTS, NB]
    T = np.ceil(C.max(axis=0) / 128).astype(np.int64)  # [SLOTS, NB]

    # edge segments in (group, bank, slot) order; et columns in
    # (group, slot, bank) order (slot-contiguous).
    seg_edge_off = np.zeros((SLOTS, NB), dtype=np.int64)  # offset in padded edge stream
    et_col = np.zeros((SLOTS, NB), dtype=np.int64)  # first et column index
    grp_gather_off = np.zeros((NGROUPS, NB), dtype=np.int64)  # edge offset of each gather
    grp_gather_sz = np.zeros((NGROUPS, NB), dtype=np.int64)  # edges per gather
    off_e = 0
    off_c = 0
    for g in range(NGROUPS):
        ss = range(g * G, min((g + 1) * G, SLOTS))
        for b in range(NB):
            grp_gather_off[g, b] = off_e
            for s in ss:
                seg_edge_off[s, b] = off_e
                off_e += T[s, b] * 128
            grp_gather_sz[g, b] = off_e - grp_gather_off[g, b]
        for s in ss:
            for b in range(NB):
                et_col[s, b] = off_c
                off_c += T[s, b]
    epad = off_e
    et_total = off_c
    return dict(
        tile_id=tile_id, bank=bank, perm=perm, core_of_tile=core_of_tile,
        slot_of_tile=slot_of_tile, T=T, seg_edge_off=seg_edge_off,
        et_col=et_col, grp_gather_off=grp_gather_off, grp_gather_sz=grp_gather_sz,
        epad=int(epad), et_total=int(et_total),
    )


def _pack_host_data(features, src, dst, W, b, gamma, beta, skip_W, skip_b, plan):
    """Build shared (replicated) and per-core input arrays."""
    T = plan["T"]
    epad, et_total = plan["epad"], plan["et_total"]

    deg_out = np.bincount(src, minlength=N).astype(np.float32)
    deg_in = np.bincount(dst, minlength=N).astype(np.float32)
    norm_out = 1.0 / np.sqrt(np.maximum(deg_out, 1.0))
    ndinv = np.sqrt(np.maximum(deg_in, 1.0))  # 1/norm_dst, for the b!=0 path

    # order edges by (core, group, bank, slot, src)
    core_e = plan["core_of_tile"][plan["tile_id"]]
    slot_e = plan["slot_of_tile"][plan["tile_id"]]
    group_e = slot_e // G
    order = np.lexsort((src, slot_e, plan["bank"], group_e, core_e))
    src_o = src[order]
    dst_o = dst[order]
    bank_o = plan["bank"][order]
    core_o = core_e[order]
    slot_o = slot_e[order]

    # rank within each (core, slot, bank) run
    E = len(src_o)
    key_change = np.ones(E, dtype=bool)
    key_change[1:] = (
        (core_o[1:] != core_o[:-1]) | (slot_o[1:] != slot_o[:-1]) | (bank_o[1:] != bank_o[:-1])
    )
    run_start = np.maximum.accumulate(np.where(key_change, np.arange(E), 0))
    rank = np.arange(E) - run_start

    pos = plan["seg_edge_off"][slot_o, bank_o] + rank  # position in padded stream
    assert (rank < T[slot_o, bank_o] * 128).all()

    idx_pad = np.zeros((NC, epad), dtype=np.int16)
    # padded (unused) edges carry slot=-1 so the 0/1 selection mask zeroes them
    slot_pad = np.full((NC, epad), -1.0, dtype=np.float16)
    idx_pad[core_o, pos] = (src_o - bank_o * BS).astype(np.int16)
    slot_pad[core_o, pos] = (dst_o - plan["perm"][core_o, slot_o] * TD).astype(np.float16)

    # wrapped int16 idx layout: per 16-edge column, replicated over 8x16 partitions
    idx_w = np.ascontiguousarray(
        np.tile(idx_pad.reshape(NC, epad // 16, 16).transpose(0, 2, 1), (1, 8, 1))
    )  # [NC, 128, epad/16]
    # slot layout: edge i -> partition i%128, et column; but the et-column order
    # differs from the edge-stream order, so map through seg offsets.
    slot_w = np.empty((NC, 128, et_total), dtype=np.float16)
    slot_seg = slot_pad.reshape(NC, epad // 128, 128).transpose(0, 2, 1)  # by edge col
    for s in range(SLOTS):
        for bk in range(NB):
            n = int(T[s, bk])
            e0 = int(plan["seg_edge_off"][s, bk]) // 128
            c0 = int(plan["et_col"][s, bk])
            slot_w[:, :, c0:c0 + n] = slot_seg[:, :, e0:e0 + n]

    # fp16 feature banks pre-scaled by norm_src (zero-padded to NP rows)
    fpad16 = np.zeros((NP, F), dtype=np.float16)
    fpad16[:N] = (features * norm_out[:, None]).astype(np.float16)
    fbanks = [np.ascontiguousarray(fpad16[k * BS:(k + 1) * BS]) for k in range(NB)]

    # per-core transposed skip features in slot order (unscaled), fp16
    rawpad16 = np.zeros((NP, F), dtype=np.float16)
    rawpad16[:N] = features.astype(np.float16)
    featT = np.empty((NC, F, SLOTS * TD), dtype=np.float16)
    ndinv_pc = np.ones((NC, 1, SLOTS * TD), dtype=np.float16)
    ndinv_pad = np.ones(NP, dtype=np.float32)
    ndinv_pad[:N] = ndinv
    for c in range(NC):
        rows = (plan["perm"][c][:, None] * TD + np.arange(TD)[None, :]).reshape(-1)
        featT[c] = rawpad16[rows].T
        ndinv_pc[c, 0] = ndinv_pad[rows].astype(np.float16)

    shared = dict(
        iota=np.ascontiguousarray(np.broadcast_to(np.arange(TD, dtype=np.float16), (128, TD))),
        Wh=W.astype(np.float16), brow=b.astype(np.float16).reshape(1, HID),
        skipW=skip_W.astype(np.float16), skipbrow=skip_b.astype(np.float32).reshape(1, HID),
        ones16=np.ones((1, 128), dtype=np.float16),
        ones32=np.ones((1, 128), dtype=np.float32),
        gammab=np.ascontiguousarray(np.broadcast_to(gamma.astype(np.float32), (128, HID))),
        betab=np.ascontiguousarray(np.broadcast_to(beta.astype(np.float32), (128, HID))),
    )
    for k in range(NB):
        shared[f"fb{k}"] = fbanks[k]

    per_core = []
    for c in range(NC):
        per_core.append(dict(
            idx=idx_w[c], slotv=slot_w[c], featT=featT[c], ndinv=ndinv_pc[c],
        ))
    return shared, per_core


# ---------------- bass program ----------------

def build_program(plan, trivial_affine, trivial_b=False, trivial_skipb=False, debug=False):
    """One SPMD program; structure depends only on plan['T'] (+ affine/bias triviality)."""
    T = plan["T"]
    epad, et_total = plan["epad"], plan["et_total"]

    nc = bacc.Bacc("TRN2", target_bir_lowering=False, debug=debug, num_swdge_queues=4)

    d_fb = [nc.dram_tensor(f"fb{k}", [BS, F], f16, kind="ExternalInput") for k in range(NB)]
    d_idx = nc.dram_tensor("idx", [128, epad // 16], i16, kind="ExternalInput")
    d_slot = nc.dram_tensor("slotv", [128, et_total], f16, kind="ExternalInput")
    d_featT = nc.dram_tensor("featT", [F, SLOTS * TD], f16, kind="ExternalInput")
    d_ndinv = nc.dram_tensor("ndinv", [1, SLOTS * TD], f16, kind="ExternalInput")
    d_iota = nc.dram_tensor("iota", [128, TD], f16, kind="ExternalInput")
    d_W = nc.dram_tensor("Wh", [F, HID], f16, kind="ExternalInput")
    d_brow = nc.dram_tensor("brow", [1, HID], f16, kind="ExternalInput")
    d_skipW = nc.dram_tensor("skipW", [F, HID], f16, kind="ExternalInput")
    d_skipbrow = nc.dram_tensor("skipbrow", [1, HID], f32, kind="ExternalInput")
    d_ones16 = nc.dram_tensor("ones16", [1, 128], f16, kind="ExternalInput")
    d_ones32 = nc.dram_tensor("ones32", [1, 128], f32, kind="ExternalInput")
    d_gammab = nc.dram_tensor("gammab", [128, HID], f32, kind="ExternalInput")
    d_betab = nc.dram_tensor("betab", [128, HID], f32, kind="ExternalInput")
    # out is [TD, SLOTS*HID]: partition-major so group stores are contiguous
    # per partition (few big descriptors); host untransposes.
    d_out = nc.dram_tensor("out", [TD, SLOTS * HID], f16, kind="ExternalOutput")

    import itertools
    qrr = itertools.cycle(range(4))  # round-robin SWDGE queue for gather chunks

    with tile.TileContext(nc) as tc:
        with (
            tc.tile_pool(name="const", bufs=1) as const,
            tc.tile_pool(name="meta", bufs=2) as meta,
            tc.tile_pool(name="hpool", bufs=2) as hpool,
            tc.tile_pool(name="spool", bufs=2) as spool,
            tc.tile_pool(name="ypool", bufs=4) as ypool,
            tc.tile_pool(name="stats", bufs=4) as stats,
            tc.tile_pool(name="opool", bufs=2) as opool,
            tc.tile_pool(name="psA", bufs=2, space="PSUM") as psA,
            tc.tile_pool(name="psG", bufs=2, space="PSUM") as psG,
            tc.tile_pool(name="psS", bufs=2, space="PSUM") as psS,
        ):
            t_iota = const.tile([128, TD], f16)
            nc.sync.dma_start(t_iota[:], d_iota[:])
            t_W = const.tile([F, HID], f16)
            nc.sync.dma_start(t_W[:], d_W[:])
            t_brow = const.tile([1, HID], f16)
            nc.sync.dma_start(t_brow[:], d_brow[:])
            t_skipW = const.tile([F, HID], f16)
            nc.sync.dma_start(t_skipW[:], d_skipW[:])
            t_ones16 = const.tile([1, 128], f16)
            nc.sync.dma_start(t_ones16[:], d_ones16[:])
            if not trivial_b:
                t_ndinv = const.tile([1, SLOTS * TD], f16)
                nc.sync.dma_start(t_ndinv[:], d_ndinv[:])
            if not trivial_skipb:
                t_skipbrow = const.tile([1, HID], f32)
                nc.sync.dma_start(t_skipbrow[:], d_skipbrow[:])
                t_ones32 = const.tile([1, 128], f32)
                nc.sync.dma_start(t_ones32[:], d_ones32[:])
            if not trivial_affine:
                t_gammab = const.tile([128, HID], f32)
                nc.sync.dma_start(t_gammab[:], d_gammab[:])
                t_betab = const.tile([128, HID], f32)
                nc.sync.dma_start(t_betab[:], d_betab[:])
            t_eps = const.tile([128, 1], f32)
            nc.vector.memset(t_eps[:], EPS)

            for g in range(NGROUPS):
                s_lo = g * G
                s_hi = min(s_lo + G, SLOTS)
                ns = s_hi - s_lo
                gt = [int(plan["grp_gather_sz"][g, b]) for b in range(NB)]
                goff = [int(plan["grp_gather_off"][g, b]) for b in range(NB)]
                c_lo = int(plan["et_col"][s_lo, 0])
                c_hi = c_lo + sum(gt) // 128

                # group metadata loads
                t_idx = meta.tile([128, sum(gt) // 16], i16, tag="idx")
                nc.sync.dma_start(t_idx[:], d_idx[:, goff[0] // 16: goff[0] // 16 + sum(gt) // 16])
                t_slot = meta.tile([128, c_hi - c_lo], f16, tag="slot")
                nc.sync.dma_start(t_slot[:], d_slot[:, c_lo:c_hi])
                t_featT = meta.tile([F, ns * TD], f16, tag="featT")
                nc.sync.dma_start(t_featT[:], d_featT[:, s_lo * TD: s_hi * TD])

                # one big gather per (group, bank), each bank on its own SWDGE
                # queue so the 4 rings' DMAs overlap
                t_H = []
                for bk in range(NB):
                    if gt[bk] == 0:
                        t_H.append(None)
                        continue
                    th = hpool.tile([128, gt[bk] // 128, F], f16, tag=f"H{bk}")
                    for ch in range(0, gt[bk], GCH):
                        sz = min(GCH, gt[bk] - ch)
                        off16 = (goff[bk] - goff[0] + ch) // 16
                        nc.gpsimd.dma_gather(
                            th[:, ch // 128: (ch + sz) // 128, :], d_fb[bk][:],
                            t_idx[:, off16: off16 + sz // 16],
                            sz, sz, F, queue_num=next(qrr),
                        )
                    t_H.append(th)

                t_out = opool.tile([128, ns, HID], f16, tag="out")

                for s in range(s_lo, s_hi):
                    n_et = int(T[s].sum())
                    # ---- selection masks for the whole slot in one DVE op ----
                    if n_et > 0:
                        c0 = int(plan["et_col"][s, 0]) - c_lo
                        t_S = spool.tile([128, n_et, TD], f16, tag="S")
                        nc.vector.tensor_tensor(
                            out=t_S[:],
                            in0=t_iota[:].unsqueeze(1).broadcast_to([128, n_et, TD]),
                            in1=t_slot[:, c0:c0 + n_et].unsqueeze(2).broadcast_to([128, n_et, TD]),
                            op=mybir.AluOpType.is_equal,
                        )
                        # ---- aggregation ----
                        t_aggT_ps = psA.tile([F, TD], f32, tag="aggT")
                        k = 0
                        for bk in range(NB):
                            h_base = (int(plan["seg_edge_off"][s, bk]) - goff[bk]) // 128
                            for e in range(int(T[s, bk])):
                                nc.tensor.matmul(
                                    out=t_aggT_ps[:],
                                    lhsT=t_H[bk][:, h_base + e, :],
                                    rhs=t_S[:, k, :],
                                    start=(k == 0), stop=(k == n_et - 1),
                                )
                                k += 1
                        t_aggT = ypool.tile([F, TD], f16, tag="aggT_sb")
                        nc.scalar.activation(
                            out=t_aggT[:], in_=t_aggT_ps[:],
                            func=mybir.ActivationFunctionType.Copy,
                        )

                    # ---- gcn = agg @ W (+ ndinv (x) b) ----
                    t_gcn_ps = psG.tile([TD, HID], f32, tag="gcn")
                    need_brow = (not trivial_b) or n_et == 0
                    if need_brow:
                        lhs_b = (
                            t_ndinv[:, s * TD:(s + 1) * TD] if not trivial_b
                            else t_ones16[:]
                        )
                        nc.tensor.matmul(
                            out=t_gcn_ps[:], lhsT=lhs_b, rhs=t_brow[:],
                            start=True, stop=(n_et == 0),
                        )
                    if n_et > 0:
                        nc.tensor.matmul(
                            out=t_gcn_ps[:], lhsT=t_aggT[:], rhs=t_W[:],
                            start=not need_brow, stop=True,
                        )

                    # ---- skip = feat @ skip_W + skip_b ----
                    t_skip_ps = psS.tile([TD, HID], f32, tag="skip")
                    if not trivial_skipb:
                        nc.tensor.matmul(
                            out=t_skip_ps[:], lhsT=t_ones32[:], rhs=t_skipbrow[:],
                            start=True, stop=False,
                        )
                    nc.tensor.matmul(
                        out=t_skip_ps[:], lhsT=t_featT[:, (s - s_lo) * TD:(s - s_lo + 1) * TD],
                        rhs=t_skipW[:], start=trivial_skipb, stop=True,
                    )

                    # ---- layernorm + relu + skip add ----
                    t_stats = stats.tile([TD, 6], f32, tag="bn")
                    nc.vector.bn_stats(out=t_stats[:], in_=t_gcn_ps[:])
                    t_mv = stats.tile([TD, 2], f32, tag="mv")
                    nc.vector.bn_aggr(out=t_mv[:], in_=t_stats[:])
                    t_std = stats.tile([TD, 1], f32, tag="std")
                    nc.scalar.activation(
                        out=t_std[:], in_=t_mv[:, 1:2],
                        func=mybir.ActivationFunctionType.Sqrt, bias=t_eps[:],
                    )
                    t_rstd = stats.tile([TD, 1], f32, tag="rstd")
                    nc.vector.reciprocal(out=t_rstd[:], in_=t_std[:])
                    t_y = ypool.tile([TD, HID], f32, tag="y")
                    nc.vector.tensor_scalar(
                        out=t_y[:], in0=t_gcn_ps[:],
                        scalar1=t_mv[:, 0:1], scalar2=t_rstd[:],
                        op0=mybir.AluOpType.subtract, op1=mybir.AluOpType.mult,
                    )
                    if not trivial_affine:
                        nc.vector.tensor_tensor(
                            out=t_y[:], in0=t_y[:], in1=t_gammab[:], op=mybir.AluOpType.mult
                        )
                        nc.vector.tensor_tensor(
                            out=t_y[:], in0=t_y[:], in1=t_betab[:], op=mybir.AluOpType.add
                        )
                    t_r = ypool.tile([TD, HID], f32, tag="r")
                    nc.scalar.activation(
                        out=t_r[:], in_=t_y[:], func=mybir.ActivationFunctionType.Relu
                    )
                    nc.vector.tensor_tensor(
                        out=t_out[:, s - s_lo, :], in0=t_r[:], in1=t_skip_ps[:],
                        op=mybir.AluOpType.add,
                    )

                nc.sync.dma_start(
                    d_out[:, s_lo * HID: s_hi * HID], t_out[:, :ns, :]
                )

    nc.compile()
    return nc


# ---------------- public entry ----------------

_CACHE = {}
_LAST = {}


def kernel(features, src, dst, W, b, gamma, beta, skip_W, skip_b):
    features = np.asarray(features, dtype=np.float32)
    src = np.asarray(src).astype(np.int64)
    dst = np.asarray(dst).astype(np.int64)
    W = np.asarray(W, dtype=np.float32)
    b = np.asarray(b, dtype=np.float32)
    gamma = np.asarray(gamma, dtype=np.float32)
    beta = np.asarray(beta, dtype=np.float32)
    skip_W = np.asarray(skip_W, dtype=np.float32)
    skip_b = np.asarray(skip_b, dtype=np.float32)

    plan = _plan(src, dst)
    shared, per_core = _pack_host_data(
        features, src, dst, W, b, gamma, beta, skip_W, skip_b, plan
    )
    trivial_affine = bool(np.all(gamma == 1.0) and np.all(beta == 0.0))
    trivial_b = bool(np.all(b == 0.0))
    trivial_skipb = bool(np.all(skip_b == 0.0))

    key = (plan["T"].tobytes(), trivial_affine, trivial_b, trivial_skipb)
    if key not in _CACHE:
        _CACHE[key] = build_program(plan, trivial_affine, trivial_b, trivial_skipb)
    nc = _CACHE[key]

    from concourse.bass_utils import run_bass_kernel_spmd

    in_maps = [{**shared, **pc} for pc in per_core]
    _LAST.update(plan=plan, nc=nc, in_maps=in_maps)
    res = run_bass_kernel_spmd(nc, in_maps, core_ids=list(range(NC)))

    out_full = np.empty((NP, HID), dtype=np.float32)
    for c in range(NC):
        oc = res.results[c]["out"].astype(np.float32).reshape(TD, SLOTS, HID)
        oc = oc.transpose(1, 0, 2)  # [SLOTS, TD, HID]
        out_full[plan["perm"][c][:, None] * TD + np.arange(TD)[None, :]] = oc
    return out_full[:N]


# revision 24
# speedup vs baseline: 1.4080x; 1.0113x over previous
"""GCN block (GraphConv + LayerNorm + ReLU + skip projection) on 8 Trainium2 cores.

Strategy (dst-node sharding, per spec sharding_hint):
- 100000 dst nodes -> 784 tiles of 128 dsts (padded to 100352); tiles snake-dealt
  to 8 cores by edge count so every core runs an identical (SPMD) program.
- Edges routed to the core owning their dst tile. Per (tile, src-bank) edge lists
  are padded to multiples of 128; the per-slot/bank edge-tile counts are made
  uniform across cores (max), so one NEFF serves all cores.
- Normalization folding: the gathered feature bank is pre-scaled by norm_src on
  the host, and norm_dst is dropped entirely — LayerNorm is invariant to a
  per-row scale, and the b!=0 case is fixed exactly by adding (1/norm_dst) (x) b
  via an outer-product matmul (LN(nd*(agg@W + b/nd)) == LN of the true gcn row).
  The S matrix is then a pure 0/1 selection mask: S[e, d] = (slot_e == d),
  built for a whole slot in ONE DVE tensor_tensor(is_equal) using stride-0
  broadcast APs (padded edges carry slot=-1 so they never match).
- Aggregation agg^T = H^T S per 128-edge tile on TensorE; H = gathered fp16
  pre-scaled src rows (dma_gather, int16 idxs => 4 feature banks of 25088 rows,
  one big gather per (group, bank) on its own SWDGE queue).
- gcn = agg @ W (+ ndinv (x) b if b!=0); LayerNorm via bn_stats/bn_aggr; skip =
  features @ skip_W (+ skip_b); relu + add; fp16 output DMA per 8-slot group.
"""

import sys

sys.path.insert(0, "/opt/trn_rl_repo")

import numpy as np

import concourse.bass as bass  # noqa: F401
import concourse.tile as tile
from concourse import bacc, mybir

# ---------------- problem constants (hardcoded per spec) ----------------
N = 100000
F = 128
HID = 256
NC = 8
TD = 128  # dsts per tile
EPS = 1e-5
NTILES = 784  # ceil(100000/128)=782, padded to a multiple of NC
NP = NTILES * TD  # 100352 padded node space
NB = 4  # src banks (dma_gather idxs are int16)
BS = NP // NB  # 25088 rows per bank
SLOTS = NTILES // NC  # 98 per core
G = 8  # slots per gather group
NGROUPS = (SLOTS + G - 1) // G  # 13
GCH = 1024  # max idxs per dma_gather (hard HW cap: 1280+ crashes the device)

f16 = mybir.dt.float16
f32 = mybir.dt.float32
i16 = mybir.dt.int16


# ---------------- host-side graph preprocessing ----------------

def _plan(src, dst, opt_rounds=4000):
    """Compute the SPMD-uniform structure: tile->core deal, per (slot, bank)
    edge-tile counts T[s][b], and the flat segment layout. Edge segments are
    laid out (group, bank, slot)-major (matching the per-bank gathers); the
    et (edge-tile) columns are laid out (group, slot, bank)-major so each
    slot's columns are contiguous for the one-shot S build.

    Tiles are grouped into slots of NC so that the per-slot/bank max (which all
    cores pad to) is small: snake-deal by total count, then local-search swaps
    minimizing sum_s,b max_c ceil(cnt/128). Deterministic (fixed iteration
    count) so repeated runs produce identical programs and hit the NEFF cache."""
    tile_id = dst // TD
    bank = src // BS

    cnt = np.zeros((NTILES, NB), dtype=np.int64)
    np.add.at(cnt, (tile_id, bank), 1)
    tot = cnt.sum(1)

    # snake-deal tiles (desc by edge count) to slot groups
    order = np.argsort(-tot, kind="stable")
    arr = np.empty((SLOTS, NC), dtype=np.int64)
    for i, t in enumerate(order):
        r, j = divmod(i, NC)
        c = j if r % 2 == 0 else NC - 1 - j
        arr[r, c] = t

    # local search: swap tiles between slot groups to reduce padded edge tiles.
    # Batched-vectorized: propose `batch` random swaps at once, score them all
    # with numpy, apply a conflict-free improving subset.
    rng = np.random.default_rng(0)
    slot_cnt = cnt[arr]  # [SLOTS, NC, NB]
    costs = np.ceil(slot_cnt.max(axis=1) / 128).astype(np.int64).sum(axis=1)
    batch = 4096
    for _ in range(opt_rounds):
        s1 = rng.integers(0, SLOTS, batch)
        s2 = rng.integers(0, SLOTS, batch)
        i1 = rng.integers(0, NC, batch)
        i2 = rng.integers(0, NC, batch)
        tie = rng.random(batch) < 0.2
        a1 = slot_cnt[s1].copy()
        a1[np.arange(batch), i1] = cnt[arr[s2, i2]]
        a2 = slot_cnt[s2].copy()
        a2[np.arange(batch), i2] = cnt[arr[s1, i1]]
        n1 = np.ceil(a1.max(axis=1) / 128).astype(np.int64).sum(axis=1)
        n2 = np.ceil(a2.max(axis=1) / 128).astype(np.int64).sum(axis=1)
        delta = (n1 + n2) - (costs[s1] + costs[s2])
        cand = np.nonzero(
            (s1 != s2) & ((delta < 0) | ((delta == 0) & tie))
        )[0]
        touched = set()
        for j in cand:
            if s1[j] in touched or s2[j] in touched:
                continue
            touched.add(int(s1[j]))
            touched.add(int(s2[j]))
            t1, t2 = arr[s1[j], i1[j]], arr[s2[j], i2[j]]
            arr[s1[j], i1[j]], arr[s2[j], i2[j]] = t2, t1
            slot_cnt[s1[j], i1[j]] = cnt[t2]
            slot_cnt[s2[j], i2[j]] = cnt[t1]
            costs[s1[j]] = np.ceil(slot_cnt[s1[j]].max(axis=0) / 128).astype(np.int64).sum()
            costs[s2[j]] = np.ceil(slot_cnt[s2[j]].max(axis=0) / 128).astype(np.int64).sum()
    perm = np.ascontiguousarray(arr.T)  # [NC, SLOTS]

    core_of_tile = np.empty(NTILES, dtype=np.int64)
    slot_of_tile = np.empty(NTILES, dtype=np.int64)
    for c in range(NC):
        core_of_tile[perm[c]] = c
        slot_of_tile[perm[c]] = np.arange(SLOTS)

    # uniform edge-tile counts: T[s][b] = max over cores
    C = cnt[perm]  # [NC, SLOTS, NB]
    T = np.ceil(C.max(axis=0) / 128).astype(np.int64)  # [SLOTS, NB]

    # edge segments in (group, bank, slot) order; et columns in
    # (group, slot, bank) order (slot-contiguous).
    seg_edge_off = np.zeros((SLOTS, NB), dtype=np.int64)  # offset in padded edge stream
    et_col = np.zeros((SLOTS, NB), dtype=np.int64)  # first et column index
    grp_gather_off = np.zeros((NGROUPS, NB), dtype=np.int64)  # edge offset of each gather
    grp_gather_sz = np.zeros((NGROUPS, NB), dtype=np.int64)  # edges per gather
    off_e = 0
    off_c = 0
    for g in range(NGROUPS):
        ss = range(g * G, min((g + 1) * G, SLOTS))
        for b in range(NB):
            grp_gather_off[g, b] = off_e
            for s in ss:
                seg_edge_off[s, b] = off_e
                off_e += T[s, b] * 128
            grp_gather_sz[g, b] = off_e - grp_gather_off[g, b]
        for s in ss:
            for b in range(NB):
                et_col[s, b] = off_c
                off_c += T[s, b]
    epad = off_e
    et_total = off_c
    return dict(
        tile_id=tile_id, bank=bank, perm=perm, core_of_tile=core_of_tile,
        slot_of_tile=slot_of_tile, T=T, seg_edge_off=seg_edge_off,
        et_col=et_col, grp_gather_off=grp_gather_off, grp_gather_sz=grp_gather_sz,
        epad=int(epad), et_total=int(et_total),
    )


def _pack_host_data(features, src, dst, W, b, gamma, beta, skip_W, skip_b, plan):
    """Build shared (replicated) and per-core input arrays."""
    T = plan["T"]
    epad, et_total = plan["epad"], plan["et_total"]

    deg_out = np.bincount(src, minlength=N).astype(np.float32)
    deg_in = np.bincount(dst, minlength=N).astype(np.float32)
    norm_out = 1.0 / np.sqrt(np.maximum(deg_out, 1.0))
    ndinv = np.sqrt(np.maximum(deg_in, 1.0))  # 1/norm_dst, for the b!=0 path

    # order edges by (core, group, bank, slot, src)
    core_e = plan["core_of_tile"][plan["tile_id"]]
    slot_e = plan["slot_of_tile"][plan["tile_id"]]
    group_e = slot_e // G
    order = np.lexsort((src, slot_e, plan["bank"], group_e, core_e))
    src_o = src[order]
    dst_o = dst[order]
    bank_o = plan["bank"][order]
    core_o = core_e[order]
    slot_o = slot_e[order]

    # rank within each (core, slot, bank) run
    E = len(src_o)
    key_change = np.ones(E, dtype=bool)
    key_change[1:] = (
        (core_o[1:] != core_o[:-1]) | (slot_o[1:] != slot_o[:-1]) | (bank_o[1:] != bank_o[:-1])
    )
    run_start = np.maximum.accumulate(np.where(key_change, np.arange(E), 0))
    rank = np.arange(E) - run_start

    pos = plan["seg_edge_off"][slot_o, bank_o] + rank  # position in padded stream
    assert (rank < T[slot_o, bank_o] * 128).all()

    idx_pad = np.zeros((NC, epad), dtype=np.int16)
    # padded (unused) edges carry slot=-1 so the 0/1 selection mask zeroes them
    slot_pad = np.full((NC, epad), -1.0, dtype=np.float16)
    idx_pad[core_o, pos] = (src_o - bank_o * BS).astype(np.int16)
    slot_pad[core_o, pos] = (dst_o - plan["perm"][core_o, slot_o] * TD).astype(np.float16)

    # wrapped int16 idx layout: per 16-edge column, replicated over 8x16 partitions
    idx_w = np.ascontiguousarray(
        np.tile(idx_pad.reshape(NC, epad // 16, 16).transpose(0, 2, 1), (1, 8, 1))
    )  # [NC, 128, epad/16]
    # slot layout: edge i -> partition i%128, et column; but the et-column order
    # differs from the edge-stream order, so map through seg offsets.
    slot_w = np.empty((NC, 128, et_total), dtype=np.float16)
    slot_seg = slot_pad.reshape(NC, epad // 128, 128).transpose(0, 2, 1)  # by edge col
    for s in range(SLOTS):
        for bk in range(NB):
            n = int(T[s, bk])
            e0 = int(plan["seg_edge_off"][s, bk]) // 128
            c0 = int(plan["et_col"][s, bk])
            slot_w[:, :, c0:c0 + n] = slot_seg[:, :, e0:e0 + n]

    # fp16 feature banks pre-scaled by norm_src (zero-padded to NP rows)
    fpad16 = np.zeros((NP, F), dtype=np.float16)
    fpad16[:N] = (features * norm_out[:, None]).astype(np.float16)
    fbanks = [np.ascontiguousarray(fpad16[k * BS:(k + 1) * BS]) for k in range(NB)]

    # per-core transposed skip features in slot order (unscaled), fp16
    rawpad16 = np.zeros((NP, F), dtype=np.float16)
    rawpad16[:N] = features.astype(np.float16)
    featT = np.empty((NC, F, SLOTS * TD), dtype=np.float16)
    ndinv_pc = np.ones((NC, 1, SLOTS * TD), dtype=np.float16)
    ndinv_pad = np.ones(NP, dtype=np.float32)
    ndinv_pad[:N] = ndinv
    for c in range(NC):
        rows = (plan["perm"][c][:, None] * TD + np.arange(TD)[None, :]).reshape(-1)
        featT[c] = rawpad16[rows].T
        ndinv_pc[c, 0] = ndinv_pad[rows].astype(np.float16)

    shared = dict(
        iota=np.ascontiguousarray(np.broadcast_to(np.arange(TD, dtype=np.float16), (128, TD))),
        Wh=W.astype(np.float16), brow=b.astype(np.float16).reshape(1, HID),
        skipW=skip_W.astype(np.float16), skipbrow=skip_b.astype(np.float32).reshape(1, HID),
        ones16=np.ones((1, 128), dtype=np.float16),
        ones32=np.ones((1, 128), dtype=np.float32),
        gammab=np.ascontiguousarray(np.broadcast_to(gamma.astype(np.float32), (128, HID))),
        betab=np.ascontiguousarray(np.broadcast_to(beta.astype(np.float32), (128, HID))),
    )
    for k in range(NB):
        shared[f"fb{k}"] = fbanks[k]

    per_core = []
    for c in range(NC):
        per_core.append(dict(
            idx=idx_w[c], slotv=slot_w[c], featT=featT[c], ndinv=ndinv_pc[c],
        ))
    return shared, per_core


# ---------------- bass program ----------------

def build_program(plan, trivial_affine, trivial_b=False, trivial_skipb=False, debug=False):
    """One SPMD program; structure depends only on plan['T'] (+ affine/bias triviality)."""
    T = plan["T"]
    epad, et_total = plan["epad"], plan["et_total"]

    nc = bacc.Bacc("TRN2", target_bir_lowering=False, debug=debug, num_swdge_queues=4)

    d_fb = [nc.dram_tensor(f"fb{k}", [BS, F], f16, kind="ExternalInput") for k in range(NB)]
    d_idx = nc.dram_tensor("idx", [128, epad // 16], i16, kind="ExternalInput")
    d_slot = nc.dram_tensor("slotv", [128, et_total], f16, kind="ExternalInput")
    d_featT = nc.dram_tensor("featT", [F, SLOTS * TD], f16, kind="ExternalInput")
    d_ndinv = nc.dram_tensor("ndinv", [1, SLOTS * TD], f16, kind="ExternalInput")
    d_iota = nc.dram_tensor("iota", [128, TD], f16, kind="ExternalInput")
    d_W = nc.dram_tensor("Wh", [F, HID], f16, kind="ExternalInput")
    d_brow = nc.dram_tensor("brow", [1, HID], f16, kind="ExternalInput")
    d_skipW = nc.dram_tensor("skipW", [F, HID], f16, kind="ExternalInput")
    d_skipbrow = nc.dram_tensor("skipbrow", [1, HID], f32, kind="ExternalInput")
    d_ones16 = nc.dram_tensor("ones16", [1, 128], f16, kind="ExternalInput")
    d_ones32 = nc.dram_tensor("ones32", [1, 128], f32, kind="ExternalInput")
    d_gammab = nc.dram_tensor("gammab", [128, HID], f32, kind="ExternalInput")
    d_betab = nc.dram_tensor("betab", [128, HID], f32, kind="ExternalInput")
    # out is [TD, SLOTS*HID]: partition-major so group stores are contiguous
    # per partition (few big descriptors); host untransposes.
    d_out = nc.dram_tensor("out", [TD, SLOTS * HID], f16, kind="ExternalOutput")

    import itertools
    qrr = itertools.cycle(range(4))  # round-robin SWDGE queue for gather chunks

    with tile.TileContext(nc) as tc:
        with (
            tc.tile_pool(name="const", bufs=1) as const,
            tc.tile_pool(name="meta", bufs=2) as meta,
            tc.tile_pool(name="hpool", bufs=2) as hpool,
            tc.tile_pool(name="spool", bufs=2) as spool,
            tc.tile_pool(name="ypool", bufs=4) as ypool,
            tc.tile_pool(name="stats", bufs=4) as stats,
            tc.tile_pool(name="opool", bufs=2) as opool,
            tc.tile_pool(name="psA", bufs=2, space="PSUM") as psA,
            tc.tile_pool(name="psG", bufs=2, space="PSUM") as psG,
            tc.tile_pool(name="psS", bufs=2, space="PSUM") as psS,
        ):
            t_iota = const.tile([128, TD], f16)
            nc.sync.dma_start(t_iota[:], d_iota[:])
            t_W = const.tile([F, HID], f16)
            nc.sync.dma_start(t_W[:], d_W[:])
            t_brow = const.tile([1, HID], f16)
            nc.sync.dma_start(t_brow[:], d_brow[:])
            t_skipW = const.tile([F, HID], f16)
            nc.sync.dma_start(t_skipW[:], d_skipW[:])
            t_ones16 = const.tile([1, 128], f16)
            nc.sync.dma_start(t_ones16[:], d_ones16[:])
            if not trivial_b:
                t_ndinv = const.tile([1, SLOTS * TD], f16)
                nc.sync.dma_start(t_ndinv[:], d_ndinv[:])
            if not trivial_skipb:
                t_skipbrow = const.tile([1, HID], f32)
                nc.sync.dma_start(t_skipbrow[:], d_skipbrow[:])
                t_ones32 = const.tile([1, 128], f32)
                nc.sync.dma_start(t_ones32[:], d_ones32[:])
            if not trivial_affine:
                t_gammab = const.tile([128, HID], f32)
                nc.sync.dma_start(t_gammab[:], d_gammab[:])
                t_betab = const.tile([128, HID], f32)
                nc.sync.dma_start(t_betab[:], d_betab[:])
            t_eps = const.tile([128, 1], f32)
            nc.vector.memset(t_eps[:], EPS)

            for g in range(NGROUPS):
                s_lo = g * G
                s_hi = min(s_lo + G, SLOTS)
                ns = s_hi - s_lo
                gt = [int(plan["grp_gather_sz"][g, b]) for b in range(NB)]
                goff = [int(plan["grp_gather_off"][g, b]) for b in range(NB)]
                c_lo = int(plan["et_col"][s_lo, 0])
                c_hi = c_lo + sum(gt) // 128

                # group metadata loads
                t_idx = meta.tile([128, sum(gt) // 16], i16, tag="idx")
                nc.sync.dma_start(t_idx[:], d_idx[:, goff[0] // 16: goff[0] // 16 + sum(gt) // 16])
                t_slot = meta.tile([128, c_hi - c_lo], f16, tag="slot")
                nc.sync.dma_start(t_slot[:], d_slot[:, c_lo:c_hi])
                t_featT = meta.tile([F, ns * TD], f16, tag="featT")
                nc.sync.dma_start(t_featT[:], d_featT[:, s_lo * TD: s_hi * TD])

                # one big gather per (group, bank), each bank on its own SWDGE
                # queue so the 4 rings' DMAs overlap
                t_H = []
                for bk in range(NB):
                    if gt[bk] == 0:
                        t_H.append(None)
                        continue
                    th = hpool.tile([128, gt[bk] // 128, F], f16, tag=f"H{bk}")
                    for ch in range(0, gt[bk], GCH):
                        sz = min(GCH, gt[bk] - ch)
                        off16 = (goff[bk] - goff[0] + ch) // 16
                        nc.gpsimd.dma_gather(
                            th[:, ch // 128: (ch + sz) // 128, :], d_fb[bk][:],
                            t_idx[:, off16: off16 + sz // 16],
                            sz, sz, F, queue_num=next(qrr),
                        )
                    t_H.append(th)

                t_out = opool.tile([128, ns, HID], f16, tag="out")

                for s in range(s_lo, s_hi):
                    n_et = int(T[s].sum())
                    # ---- selection masks for the whole slot in one DVE op ----
                    if n_et > 0:
                        c0 = int(plan["et_col"][s, 0]) - c_lo
                        t_S = spool.tile([128, n_et, TD], f16, tag="S")
                        nc.vector.tensor_tensor(
                            out=t_S[:],
                            in0=t_iota[:].unsqueeze(1).broadcast_to([128, n_et, TD]),
                            in1=t_slot[:, c0:c0 + n_et].unsqueeze(2).broadcast_to([128, n_et, TD]),
                            op=mybir.AluOpType.is_equal,
                        )
                        # ---- aggregation ----
                        t_aggT_ps = psA.tile([F, TD], f32, tag="aggT")
                        k = 0
                        for bk in range(NB):
                            h_base = (int(plan["seg_edge_off"][s, bk]) - goff[bk]) // 128
                            for e in range(int(T[s, bk])):
                                nc.tensor.matmul(
                                    out=t_aggT_ps[:],
                                    lhsT=t_H[bk][:, h_base + e, :],
                                    rhs=t_S[:, k, :],
                                    start=(k == 0), stop=(k == n_et - 1),
                                )
                                k += 1
                        t_aggT = ypool.tile([F, TD], f16, tag="aggT_sb")
                        nc.scalar.activation(
                            out=t_aggT[:], in_=t_aggT_ps[:],
                            func=mybir.ActivationFunctionType.Copy,
                        )

                    # ---- gcn = agg @ W (+ ndinv (x) b) ----
                    t_gcn_ps = psG.tile([TD, HID], f32, tag="gcn")
                    need_brow = (not trivial_b) or n_et == 0
                    if need_brow:
                        lhs_b = (
                            t_ndinv[:, s * TD:(s + 1) * TD] if not trivial_b
                            else t_ones16[:]
                        )
                        nc.tensor.matmul(
                            out=t_gcn_ps[:], lhsT=lhs_b, rhs=t_brow[:],
                            start=True, stop=(n_et == 0),
                        )
                    if n_et > 0:
                        nc.tensor.matmul(
                            out=t_gcn_ps[:], lhsT=t_aggT[:], rhs=t_W[:],
                            start=not need_brow, stop=True,
                        )

                    # ---- skip = feat @ skip_W + skip_b ----
                    t_skip_ps = psS.tile([TD, HID], f32, tag="skip")
                    if not trivial_skipb:
                        nc.tensor.matmul(
                            out=t_skip_ps[:], lhsT=t_ones32[:], rhs=t_skipbrow[:],
                            start=True, stop=False,
                        )
                    nc.tensor.matmul(
                        out=t_skip_ps[:], lhsT=t_featT[:, (s - s_lo) * TD:(s - s_lo + 1) * TD],
                        rhs=t_skipW[:], start=trivial_skipb, stop=True,
                    )

                    # ---- layernorm + relu + skip add ----
                    t_stats = stats.tile([TD, 6], f32, tag="bn")
                    nc.vector.bn_stats(out=t_stats[:], in_=t_gcn_ps[:])
                    t_mv = stats.tile([TD, 2], f32, tag="mv")
                    nc.vector.bn_aggr(out=t_mv[:], in_=t_stats[:])
                    t_std = stats.tile([TD, 1], f32, tag="std")
                    nc.scalar.activation(
                        out=t_std[:], in_=t_mv[:, 1:2],
                        func=mybir.ActivationFunctionType.Sqrt, bias=t_eps[:],
                    )
                    t_rstd = stats.tile([TD, 1], f32, tag="rstd")
                    nc.vector.reciprocal(out=t_rstd[:], in_=t_std[:])
                    t_y = ypool.tile([TD, HID], f32, tag="y")
                    nc.vector.tensor_scalar(
                        out=t_y[:], in0=t_gcn_ps[:],
                        scalar1=t_mv[:, 0:1], scalar2=t_rstd[:],
                        op0=mybir.AluOpType.subtract, op1=mybir.AluOpType.mult,
                    )
                    if not trivial_affine:
                        nc.vector.tensor_tensor(
                            out=t_y[:], in0=t_y[:], in1=t_gammab[:], op=mybir.AluOpType.mult
                        )
                        nc.vector.tensor_tensor(
                            out=t_y[:], in0=t_y[:], in1=t_betab[:], op=mybir.AluOpType.add
                        )
                    # fused relu + skip add: out = max(y, 0) + skip
                    nc.vector.scalar_tensor_tensor(
                        out=t_out[:, s - s_lo, :], in0=t_y[:], scalar=0.0,
                        in1=t_skip_ps[:],
                        op0=mybir.AluOpType.max, op1=mybir.AluOpType.add,
                    )

                nc.sync.dma_start(
                    d_out[:, s_lo * HID: s_hi * HID], t_out[:, :ns, :]
                )

    nc.compile()
    return nc


# ---------------- public entry ----------------

_CACHE = {}
_LAST = {}


def kernel(features, src, dst, W, b, gamma, beta, skip_W, skip_b):
    features = np.asarray(features, dtype=np.float32)
    src = np.asarray(src).astype(np.int64)
    dst = np.asarray(dst).astype(np.int64)
    W = np.asarray(W, dtype=np.float32)
    b = np.asarray(b, dtype=np.float32)
    gamma = np.asarray(gamma, dtype=np.float32)
    beta = np.asarray(beta, dtype=np.float32)
    skip_W = np.asarray(skip_W, dtype=np.float32)
    skip_b = np.asarray(skip_b, dtype=np.float32)

    plan = _plan(src, dst)
    shared, per_core = _pack_host_data(
        features, src, dst, W, b, gamma, beta, skip_W, skip_b, plan
    )
    trivial_affine = bool(np.all(gamma == 1.0) and np.all(beta == 0.0))
    trivial_b = bool(np.all(b == 0.0))
    trivial_skipb = bool(np.all(skip_b == 0.0))

    key = (plan["T"].tobytes(), trivial_affine, trivial_b, trivial_skipb)
    if key not in _CACHE:
        _CACHE[key] = build_program(plan, trivial_affine, trivial_b, trivial_skipb)
    nc = _CACHE[key]

    from concourse.bass_utils import run_bass_kernel_spmd

    in_maps = [{**shared, **pc} for pc in per_core]
    _LAST.update(plan=plan, nc=nc, in_maps=in_maps)
    res = run_bass_kernel_spmd(nc, in_maps, core_ids=list(range(NC)))

    out_full = np.empty((NP, HID), dtype=np.float32)
    for c in range(NC):
        oc = res.results[c]["out"].astype(np.float32).reshape(TD, SLOTS, HID)
        oc = oc.transpose(1, 0, 2)  # [SLOTS, TD, HID]
        out_full[plan["perm"][c][:, None] * TD + np.arange(TD)[None, :]] = oc
    return out_full[:N]


# revision 25
# speedup vs baseline: 1.5678x; 1.1135x over previous
"""GCN block (GraphConv + LayerNorm + ReLU + skip projection) on 8 Trainium2 cores.

Strategy (dst-node sharding, per spec sharding_hint):
- 100000 dst nodes -> 784 tiles of 128 dsts (padded to 100352); tiles snake-dealt
  to 8 cores by edge count so every core runs an identical (SPMD) program.
- Edges routed to the core owning their dst tile. Per (tile, src-bank) edge lists
  are padded to multiples of 128; the per-slot/bank edge-tile counts are made
  uniform across cores (max), so one NEFF serves all cores.
- Normalization folding: the gathered feature bank is pre-scaled by norm_src on
  the host, and norm_dst is dropped entirely — LayerNorm is invariant to a
  per-row scale, and the b!=0 case is fixed exactly by adding (1/norm_dst) (x) b
  via an outer-product matmul (LN(nd*(agg@W + b/nd)) == LN of the true gcn row).
  The S matrix is then a pure 0/1 selection mask: S[e, d] = (slot_e == d),
  built for a whole slot in ONE DVE tensor_tensor(is_equal) using stride-0
  broadcast APs (padded edges carry slot=-1 so they never match).
- Aggregation agg^T = H^T S per 128-edge tile on TensorE; H = gathered fp16
  pre-scaled src rows (dma_gather, int16 idxs => 4 feature banks of 25088 rows,
  one big gather per (group, bank) on its own SWDGE queue).
- gcn = agg @ W (+ ndinv (x) b if b!=0); LayerNorm via bn_stats/bn_aggr; skip =
  features @ skip_W (+ skip_b); relu + add; fp16 output DMA per 8-slot group.
"""

import sys

sys.path.insert(0, "/opt/trn_rl_repo")

import numpy as np

import concourse.bass as bass  # noqa: F401
import concourse.tile as tile
from concourse import bacc, mybir

# ---------------- problem constants (hardcoded per spec) ----------------
N = 100000
F = 128
HID = 256
NC = 8
TD = 128  # dsts per tile
EPS = 1e-5
NTILES = 784  # ceil(100000/128)=782, padded to a multiple of NC
NP = NTILES * TD  # 100352 padded node space
NB = 4  # src banks (dma_gather idxs are int16)
BS = NP // NB  # 25088 rows per bank
SLOTS = NTILES // NC  # 98 per core
G = 8  # slots per gather group
NGROUPS = (SLOTS + G - 1) // G  # 13
GCH = 896  # 57 descs/lane: 2 chunks fit the 128-desc ring -> deeper DMA pipelining

f16 = mybir.dt.float16
f32 = mybir.dt.float32
i16 = mybir.dt.int16


# ---------------- host-side graph preprocessing ----------------

def _plan(src, dst, opt_rounds=4000):
    """Compute the SPMD-uniform structure: tile->core deal, per (slot, bank)
    edge-tile counts T[s][b], and the flat segment layout. Edge segments are
    laid out (group, bank, slot)-major (matching the per-bank gathers); the
    et (edge-tile) columns are laid out (group, slot, bank)-major so each
    slot's columns are contiguous for the one-shot S build.

    Tiles are grouped into slots of NC so that the per-slot/bank max (which all
    cores pad to) is small: snake-deal by total count, then local-search swaps
    minimizing sum_s,b max_c ceil(cnt/128). Deterministic (fixed iteration
    count) so repeated runs produce identical programs and hit the NEFF cache."""
    tile_id = dst // TD
    bank = src // BS

    cnt = np.zeros((NTILES, NB), dtype=np.int64)
    np.add.at(cnt, (tile_id, bank), 1)
    tot = cnt.sum(1)

    # snake-deal tiles (desc by edge count) to slot groups
    order = np.argsort(-tot, kind="stable")
    arr = np.empty((SLOTS, NC), dtype=np.int64)
    for i, t in enumerate(order):
        r, j = divmod(i, NC)
        c = j if r % 2 == 0 else NC - 1 - j
        arr[r, c] = t

    # local search: swap tiles between slot groups to reduce padded edge tiles.
    # Batched-vectorized: propose `batch` random swaps at once, score them all
    # with numpy, apply a conflict-free improving subset.
    rng = np.random.default_rng(0)
    slot_cnt = cnt[arr]  # [SLOTS, NC, NB]
    costs = np.ceil(slot_cnt.max(axis=1) / 128).astype(np.int64).sum(axis=1)
    batch = 4096
    for _ in range(opt_rounds):
        s1 = rng.integers(0, SLOTS, batch)
        s2 = rng.integers(0, SLOTS, batch)
        i1 = rng.integers(0, NC, batch)
        i2 = rng.integers(0, NC, batch)
        tie = rng.random(batch) < 0.2
        a1 = slot_cnt[s1].copy()
        a1[np.arange(batch), i1] = cnt[arr[s2, i2]]
        a2 = slot_cnt[s2].copy()
        a2[np.arange(batch), i2] = cnt[arr[s1, i1]]
        n1 = np.ceil(a1.max(axis=1) / 128).astype(np.int64).sum(axis=1)
        n2 = np.ceil(a2.max(axis=1) / 128).astype(np.int64).sum(axis=1)
        delta = (n1 + n2) - (costs[s1] + costs[s2])
        cand = np.nonzero(
            (s1 != s2) & ((delta < 0) | ((delta == 0) & tie))
        )[0]
        touched = set()
        for j in cand:
            if s1[j] in touched or s2[j] in touched:
                continue
            touched.add(int(s1[j]))
            touched.add(int(s2[j]))
            t1, t2 = arr[s1[j], i1[j]], arr[s2[j], i2[j]]
            arr[s1[j], i1[j]], arr[s2[j], i2[j]] = t2, t1
            slot_cnt[s1[j], i1[j]] = cnt[t2]
            slot_cnt[s2[j], i2[j]] = cnt[t1]
            costs[s1[j]] = np.ceil(slot_cnt[s1[j]].max(axis=0) / 128).astype(np.int64).sum()
            costs[s2[j]] = np.ceil(slot_cnt[s2[j]].max(axis=0) / 128).astype(np.int64).sum()
    perm = np.ascontiguousarray(arr.T)  # [NC, SLOTS]

    core_of_tile = np.empty(NTILES, dtype=np.int64)
    slot_of_tile = np.empty(NTILES, dtype=np.int64)
    for c in range(NC):
        core_of_tile[perm[c]] = c
        slot_of_tile[perm[c]] = np.arange(SLOTS)

    # uniform edge-tile counts: T[s][b] = max over cores
    C = cnt[perm]  # [NC, SLOTS, NB]
    T = np.ceil(C.max(axis=0) / 128).astype(np.int64)  # [SLOTS, NB]

    # edge segments in (group, bank, slot) order; et columns in
    # (group, slot, bank) order (slot-contiguous).
    seg_edge_off = np.zeros((SLOTS, NB), dtype=np.int64)  # offset in padded edge stream
    et_col = np.zeros((SLOTS, NB), dtype=np.int64)  # first et column index
    grp_gather_off = np.zeros((NGROUPS, NB), dtype=np.int64)  # edge offset of each gather
    grp_gather_sz = np.zeros((NGROUPS, NB), dtype=np.int64)  # edges per gather
    off_e = 0
    off_c = 0
    for g in range(NGROUPS):
        ss = range(g * G, min((g + 1) * G, SLOTS))
        for b in range(NB):
            grp_gather_off[g, b] = off_e
            for s in ss:
                seg_edge_off[s, b] = off_e
                off_e += T[s, b] * 128
            grp_gather_sz[g, b] = off_e - grp_gather_off[g, b]
        for s in ss:
            for b in range(NB):
                et_col[s, b] = off_c
                off_c += T[s, b]
    epad = off_e
    et_total = off_c
    return dict(
        tile_id=tile_id, bank=bank, perm=perm, core_of_tile=core_of_tile,
        slot_of_tile=slot_of_tile, T=T, seg_edge_off=seg_edge_off,
        et_col=et_col, grp_gather_off=grp_gather_off, grp_gather_sz=grp_gather_sz,
        epad=int(epad), et_total=int(et_total),
    )


def _pack_host_data(features, src, dst, W, b, gamma, beta, skip_W, skip_b, plan):
    """Build shared (replicated) and per-core input arrays."""
    T = plan["T"]
    epad, et_total = plan["epad"], plan["et_total"]

    deg_out = np.bincount(src, minlength=N).astype(np.float32)
    deg_in = np.bincount(dst, minlength=N).astype(np.float32)
    norm_out = 1.0 / np.sqrt(np.maximum(deg_out, 1.0))
    ndinv = np.sqrt(np.maximum(deg_in, 1.0))  # 1/norm_dst, for the b!=0 path

    # order edges by (core, group, bank, slot, src)
    core_e = plan["core_of_tile"][plan["tile_id"]]
    slot_e = plan["slot_of_tile"][plan["tile_id"]]
    group_e = slot_e // G
    order = np.lexsort((src, slot_e, plan["bank"], group_e, core_e))
    src_o = src[order]
    dst_o = dst[order]
    bank_o = plan["bank"][order]
    core_o = core_e[order]
    slot_o = slot_e[order]

    # rank within each (core, slot, bank) run
    E = len(src_o)
    key_change = np.ones(E, dtype=bool)
    key_change[1:] = (
        (core_o[1:] != core_o[:-1]) | (slot_o[1:] != slot_o[:-1]) | (bank_o[1:] != bank_o[:-1])
    )
    run_start = np.maximum.accumulate(np.where(key_change, np.arange(E), 0))
    rank = np.arange(E) - run_start

    pos = plan["seg_edge_off"][slot_o, bank_o] + rank  # position in padded stream
    assert (rank < T[slot_o, bank_o] * 128).all()

    idx_pad = np.zeros((NC, epad), dtype=np.int16)
    # padded (unused) edges carry slot=-1 so the 0/1 selection mask zeroes them
    slot_pad = np.full((NC, epad), -1.0, dtype=np.float16)
    idx_pad[core_o, pos] = (src_o - bank_o * BS).astype(np.int16)
    slot_pad[core_o, pos] = (dst_o - plan["perm"][core_o, slot_o] * TD).astype(np.float16)

    # wrapped int16 idx layout: per 16-edge column, replicated over 8x16 partitions
    idx_w = np.ascontiguousarray(
        np.tile(idx_pad.reshape(NC, epad // 16, 16).transpose(0, 2, 1), (1, 8, 1))
    )  # [NC, 128, epad/16]
    # slot layout: edge i -> partition i%128, et column; but the et-column order
    # differs from the edge-stream order, so map through seg offsets.
    slot_w = np.empty((NC, 128, et_total), dtype=np.float16)
    slot_seg = slot_pad.reshape(NC, epad // 128, 128).transpose(0, 2, 1)  # by edge col
    for s in range(SLOTS):
        for bk in range(NB):
            n = int(T[s, bk])
            e0 = int(plan["seg_edge_off"][s, bk]) // 128
            c0 = int(plan["et_col"][s, bk])
            slot_w[:, :, c0:c0 + n] = slot_seg[:, :, e0:e0 + n]

    # fp16 feature banks pre-scaled by norm_src (zero-padded to NP rows)
    fpad16 = np.zeros((NP, F), dtype=np.float16)
    fpad16[:N] = (features * norm_out[:, None]).astype(np.float16)
    fbanks = [np.ascontiguousarray(fpad16[k * BS:(k + 1) * BS]) for k in range(NB)]

    # per-core transposed skip features in slot order (unscaled), fp16
    rawpad16 = np.zeros((NP, F), dtype=np.float16)
    rawpad16[:N] = features.astype(np.float16)
    featT = np.empty((NC, F, SLOTS * TD), dtype=np.float16)
    ndinv_pc = np.ones((NC, 1, SLOTS * TD), dtype=np.float16)
    ndinv_pad = np.ones(NP, dtype=np.float32)
    ndinv_pad[:N] = ndinv
    for c in range(NC):
        rows = (plan["perm"][c][:, None] * TD + np.arange(TD)[None, :]).reshape(-1)
        featT[c] = rawpad16[rows].T
        ndinv_pc[c, 0] = ndinv_pad[rows].astype(np.float16)

    shared = dict(
        iota=np.ascontiguousarray(np.broadcast_to(np.arange(TD, dtype=np.float16), (128, TD))),
        Wh=W.astype(np.float16), brow=b.astype(np.float16).reshape(1, HID),
        skipW=skip_W.astype(np.float16), skipbrow=skip_b.astype(np.float32).reshape(1, HID),
        ones16=np.ones((1, 128), dtype=np.float16),
        ones32=np.ones((1, 128), dtype=np.float32),
        gammab=np.ascontiguousarray(np.broadcast_to(gamma.astype(np.float32), (128, HID))),
        betab=np.ascontiguousarray(np.broadcast_to(beta.astype(np.float32), (128, HID))),
    )
    for k in range(NB):
        shared[f"fb{k}"] = fbanks[k]

    per_core = []
    for c in range(NC):
        per_core.append(dict(
            idx=idx_w[c], slotv=slot_w[c], featT=featT[c], ndinv=ndinv_pc[c],
        ))
    return shared, per_core


# ---------------- bass program ----------------

def build_program(plan, trivial_affine, trivial_b=False, trivial_skipb=False, debug=False):
    """One SPMD program; structure depends only on plan['T'] (+ affine/bias triviality)."""
    T = plan["T"]
    epad, et_total = plan["epad"], plan["et_total"]

    nc = bacc.Bacc("TRN2", target_bir_lowering=False, debug=debug, num_swdge_queues=4)

    d_fb = [nc.dram_tensor(f"fb{k}", [BS, F], f16, kind="ExternalInput") for k in range(NB)]
    d_idx = nc.dram_tensor("idx", [128, epad // 16], i16, kind="ExternalInput")
    d_slot = nc.dram_tensor("slotv", [128, et_total], f16, kind="ExternalInput")
    d_featT = nc.dram_tensor("featT", [F, SLOTS * TD], f16, kind="ExternalInput")
    d_ndinv = nc.dram_tensor("ndinv", [1, SLOTS * TD], f16, kind="ExternalInput")
    d_iota = nc.dram_tensor("iota", [128, TD], f16, kind="ExternalInput")
    d_W = nc.dram_tensor("Wh", [F, HID], f16, kind="ExternalInput")
    d_brow = nc.dram_tensor("brow", [1, HID], f16, kind="ExternalInput")
    d_skipW = nc.dram_tensor("skipW", [F, HID], f16, kind="ExternalInput")
    d_skipbrow = nc.dram_tensor("skipbrow", [1, HID], f32, kind="ExternalInput")
    d_ones16 = nc.dram_tensor("ones16", [1, 128], f16, kind="ExternalInput")
    d_ones32 = nc.dram_tensor("ones32", [1, 128], f32, kind="ExternalInput")
    d_gammab = nc.dram_tensor("gammab", [128, HID], f32, kind="ExternalInput")
    d_betab = nc.dram_tensor("betab", [128, HID], f32, kind="ExternalInput")
    # out is [TD, SLOTS*HID]: partition-major so group stores are contiguous
    # per partition (few big descriptors); host untransposes.
    d_out = nc.dram_tensor("out", [TD, SLOTS * HID], f16, kind="ExternalOutput")

    import itertools
    qrr = itertools.cycle(range(4))  # round-robin SWDGE queue for gather chunks

    with tile.TileContext(nc) as tc:
        with (
            tc.tile_pool(name="const", bufs=1) as const,
            tc.tile_pool(name="meta", bufs=2) as meta,
            tc.tile_pool(name="hpool", bufs=2) as hpool,
            tc.tile_pool(name="spool", bufs=2) as spool,
            tc.tile_pool(name="ypool", bufs=4) as ypool,
            tc.tile_pool(name="stats", bufs=4) as stats,
            tc.tile_pool(name="opool", bufs=2) as opool,
            tc.tile_pool(name="psA", bufs=2, space="PSUM") as psA,
            tc.tile_pool(name="psG", bufs=2, space="PSUM") as psG,
            tc.tile_pool(name="psS", bufs=2, space="PSUM") as psS,
        ):
            t_iota = const.tile([128, TD], f16)
            nc.sync.dma_start(t_iota[:], d_iota[:])
            t_W = const.tile([F, HID], f16)
            nc.sync.dma_start(t_W[:], d_W[:])
            t_brow = const.tile([1, HID], f16)
            nc.sync.dma_start(t_brow[:], d_brow[:])
            t_skipW = const.tile([F, HID], f16)
            nc.sync.dma_start(t_skipW[:], d_skipW[:])
            t_ones16 = const.tile([1, 128], f16)
            nc.sync.dma_start(t_ones16[:], d_ones16[:])
            if not trivial_b:
                t_ndinv = const.tile([1, SLOTS * TD], f16)
                nc.sync.dma_start(t_ndinv[:], d_ndinv[:])
            if not trivial_skipb:
                t_skipbrow = const.tile([1, HID], f32)
                nc.sync.dma_start(t_skipbrow[:], d_skipbrow[:])
                t_ones32 = const.tile([1, 128], f32)
                nc.sync.dma_start(t_ones32[:], d_ones32[:])
            if not trivial_affine:
                t_gammab = const.tile([128, HID], f32)
                nc.sync.dma_start(t_gammab[:], d_gammab[:])
                t_betab = const.tile([128, HID], f32)
                nc.sync.dma_start(t_betab[:], d_betab[:])
            t_eps = const.tile([128, 1], f32)
            nc.vector.memset(t_eps[:], EPS)

            for g in range(NGROUPS):
                s_lo = g * G
                s_hi = min(s_lo + G, SLOTS)
                ns = s_hi - s_lo
                gt = [int(plan["grp_gather_sz"][g, b]) for b in range(NB)]
                goff = [int(plan["grp_gather_off"][g, b]) for b in range(NB)]
                c_lo = int(plan["et_col"][s_lo, 0])
                c_hi = c_lo + sum(gt) // 128

                # group metadata loads
                t_idx = meta.tile([128, sum(gt) // 16], i16, tag="idx")
                nc.sync.dma_start(t_idx[:], d_idx[:, goff[0] // 16: goff[0] // 16 + sum(gt) // 16])
                t_slot = meta.tile([128, c_hi - c_lo], f16, tag="slot")
                nc.sync.dma_start(t_slot[:], d_slot[:, c_lo:c_hi])
                t_featT = meta.tile([F, ns * TD], f16, tag="featT")
                nc.sync.dma_start(t_featT[:], d_featT[:, s_lo * TD: s_hi * TD])

                # one big gather per (group, bank), each bank on its own SWDGE
                # queue so the 4 rings' DMAs overlap
                t_H = []
                for bk in range(NB):
                    if gt[bk] == 0:
                        t_H.append(None)
                        continue
                    th = hpool.tile([128, gt[bk] // 128, F], f16, tag=f"H{bk}")
                    for ch in range(0, gt[bk], GCH):
                        sz = min(GCH, gt[bk] - ch)
                        off16 = (goff[bk] - goff[0] + ch) // 16
                        nc.gpsimd.dma_gather(
                            th[:, ch // 128: (ch + sz) // 128, :], d_fb[bk][:],
                            t_idx[:, off16: off16 + sz // 16],
                            sz, sz, F, queue_num=next(qrr),
                        )
                    t_H.append(th)

                t_out = opool.tile([128, ns, HID], f16, tag="out")

                for s in range(s_lo, s_hi):
                    n_et = int(T[s].sum())
                    # ---- selection masks for the whole slot in one DVE op ----
                    if n_et > 0:
                        c0 = int(plan["et_col"][s, 0]) - c_lo
                        t_S = spool.tile([128, n_et, TD], f16, tag="S")
                        nc.vector.tensor_tensor(
                            out=t_S[:],
                            in0=t_iota[:].unsqueeze(1).broadcast_to([128, n_et, TD]),
                            in1=t_slot[:, c0:c0 + n_et].unsqueeze(2).broadcast_to([128, n_et, TD]),
                            op=mybir.AluOpType.is_equal,
                        )
                        # ---- aggregation ----
                        t_aggT_ps = psA.tile([F, TD], f32, tag="aggT")
                        k = 0
                        for bk in range(NB):
                            h_base = (int(plan["seg_edge_off"][s, bk]) - goff[bk]) // 128
                            for e in range(int(T[s, bk])):
                                nc.tensor.matmul(
                                    out=t_aggT_ps[:],
                                    lhsT=t_H[bk][:, h_base + e, :],
                                    rhs=t_S[:, k, :],
                                    start=(k == 0), stop=(k == n_et - 1),
                                )
                                k += 1
                        t_aggT = ypool.tile([F, TD], f16, tag="aggT_sb")
                        nc.scalar.activation(
                            out=t_aggT[:], in_=t_aggT_ps[:],
                            func=mybir.ActivationFunctionType.Copy,
                        )

                    # ---- gcn = agg @ W (+ ndinv (x) b) ----
                    t_gcn_ps = psG.tile([TD, HID], f32, tag="gcn")
                    need_brow = (not trivial_b) or n_et == 0
                    if need_brow:
                        lhs_b = (
                            t_ndinv[:, s * TD:(s + 1) * TD] if not trivial_b
                            else t_ones16[:]
                        )
                        nc.tensor.matmul(
                            out=t_gcn_ps[:], lhsT=lhs_b, rhs=t_brow[:],
                            start=True, stop=(n_et == 0),
                        )
                    if n_et > 0:
                        nc.tensor.matmul(
                            out=t_gcn_ps[:], lhsT=t_aggT[:], rhs=t_W[:],
                            start=not need_brow, stop=True,
                        )

                    # ---- skip = feat @ skip_W + skip_b ----
                    t_skip_ps = psS.tile([TD, HID], f32, tag="skip")
                    if not trivial_skipb:
                        nc.tensor.matmul(
                            out=t_skip_ps[:], lhsT=t_ones32[:], rhs=t_skipbrow[:],
                            start=True, stop=False,
                        )
                    nc.tensor.matmul(
                        out=t_skip_ps[:], lhsT=t_featT[:, (s - s_lo) * TD:(s - s_lo + 1) * TD],
                        rhs=t_skipW[:], start=trivial_skipb, stop=True,
                    )

                    # ---- layernorm + relu + skip add ----
                    t_stats = stats.tile([TD, 6], f32, tag="bn")
                    nc.vector.bn_stats(out=t_stats[:], in_=t_gcn_ps[:])
                    t_mv = stats.tile([TD, 2], f32, tag="mv")
                    nc.vector.bn_aggr(out=t_mv[:], in_=t_stats[:])
                    t_std = stats.tile([TD, 1], f32, tag="std")
                    nc.scalar.activation(
                        out=t_std[:], in_=t_mv[:, 1:2],
                        func=mybir.ActivationFunctionType.Sqrt, bias=t_eps[:],
                    )
                    t_rstd = stats.tile([TD, 1], f32, tag="rstd")
                    nc.vector.reciprocal(out=t_rstd[:], in_=t_std[:])
                    t_y = ypool.tile([TD, HID], f32, tag="y")
                    nc.vector.tensor_scalar(
                        out=t_y[:], in0=t_gcn_ps[:],
                        scalar1=t_mv[:, 0:1], scalar2=t_rstd[:],
                        op0=mybir.AluOpType.subtract, op1=mybir.AluOpType.mult,
                    )
                    if not trivial_affine:
                        nc.vector.tensor_tensor(
                            out=t_y[:], in0=t_y[:], in1=t_gammab[:], op=mybir.AluOpType.mult
                        )
                        nc.vector.tensor_tensor(
                            out=t_y[:], in0=t_y[:], in1=t_betab[:], op=mybir.AluOpType.add
                        )
                    # fused relu + skip add: out = max(y, 0) + skip
                    nc.vector.scalar_tensor_tensor(
                        out=t_out[:, s - s_lo, :], in0=t_y[:], scalar=0.0,
                        in1=t_skip_ps[:],
                        op0=mybir.AluOpType.max, op1=mybir.AluOpType.add,
                    )

                nc.sync.dma_start(
                    d_out[:, s_lo * HID: s_hi * HID], t_out[:, :ns, :]
                )

    nc.compile()
    return nc


# ---------------- public entry ----------------

_CACHE = {}
_LAST = {}


def kernel(features, src, dst, W, b, gamma, beta, skip_W, skip_b):
    features = np.asarray(features, dtype=np.float32)
    src = np.asarray(src).astype(np.int64)
    dst = np.asarray(dst).astype(np.int64)
    W = np.asarray(W, dtype=np.float32)
    b = np.asarray(b, dtype=np.float32)
    gamma = np.asarray(gamma, dtype=np.float32)
    beta = np.asarray(beta, dtype=np.float32)
    skip_W = np.asarray(skip_W, dtype=np.float32)
    skip_b = np.asarray(skip_b, dtype=np.float32)

    plan = _plan(src, dst)
    shared, per_core = _pack_host_data(
        features, src, dst, W, b, gamma, beta, skip_W, skip_b, plan
    )
    trivial_affine = bool(np.all(gamma == 1.0) and np.all(beta == 0.0))
    trivial_b = bool(np.all(b == 0.0))
    trivial_skipb = bool(np.all(skip_b == 0.0))

    key = (plan["T"].tobytes(), trivial_affine, trivial_b, trivial_skipb)
    if key not in _CACHE:
        _CACHE[key] = build_program(plan, trivial_affine, trivial_b, trivial_skipb)
    nc = _CACHE[key]

    from concourse.bass_utils import run_bass_kernel_spmd

    in_maps = [{**shared, **pc} for pc in per_core]
    _LAST.update(plan=plan, nc=nc, in_maps=in_maps)
    res = run_bass_kernel_spmd(nc, in_maps, core_ids=list(range(NC)))

    out_full = np.empty((NP, HID), dtype=np.float32)
    for c in range(NC):
        oc = res.results[c]["out"].astype(np.float32).reshape(TD, SLOTS, HID)
        oc = oc.transpose(1, 0, 2)  # [SLOTS, TD, HID]
        out_full[plan["perm"][c][:, None] * TD + np.arange(TD)[None, :]] = oc
    return out_full[:N]
